# revision 1
# baseline (speedup 1.0000x reference)
"""Trainium2 Bass kernel for nn_BertClassifier_77309411685.

Data-parallel over 8 NeuronCores: each core handles 256 samples of the
2048-sample batch; the small base linear and 12 expert heads are replicated.

Per-core device algorithm (B_c=256 samples, processed as 2 halves of 128):
  1. indirect-DMA gather: for each sample, the 8 consecutive embedding rows
     starting at `start` (spans are 1..8 long and start <= S-9, so 8 rows are
     always in bounds), split into two 4-row chunks so the masked-mean can
     start while the second chunk is still in flight. One sample per
     partition.
  2. masked mean over the span via fused DVE multiply-accumulate with
     per-partition weights (i < len) / len.
  3. two static context rows loaded with strided DMA.
  4. PE transposes build featT [3H=2304, 256] (feature-major) from the
     sample-major center/context tiles; 3 transposes share one PSUM bank and
     drain with a single strided copy.
  5. base linear: hiddenT[inner, b] = relu(W_base @ feat + b_base) via 18
     K-chunk matmuls per 128-wide inner tile, bias+relu fused in the
     PSUM->SBUF activation.
  6. expert heads: compute all 12 experts at once, out36[b, e*3+n], with the
     bias folded in as an extra K=1 matmul against a ones row; then select
     the right expert per sample with an is_equal mask and a strided reduce.

Constants (identity matrix, iota ramps, per-partition row bases) are shipped
as one small DRAM input instead of being built with gpsimd ops on device.
"""

import numpy as np
from contextlib import ExitStack

import concourse.bass as bass
import concourse.tile as tile
from concourse import bacc, mybir
from concourse.bass import IndirectOffsetOnAxis
from concourse.bass_utils import run_bass_kernel_spmd

F32 = mybir.dt.float32
I32 = mybir.dt.int32

B, S, H = 2048, 256, 768
INNER, NB_CTX, NB_EXPERTS, NB_LABELS = 256, 2, 12, 3
NCORES = 8
BC = B // NCORES           # 256 samples per core
F3H = (NB_CTX + 1) * H     # 2304
KC = F3H // 128            # 18 contraction chunks
NE = NB_EXPERTS * NB_LABELS  # 36
SPAN = 8                   # max span length; always safe to gather 8 rows
HC = H // 128              # 6 h-chunks per feature block

# The reference picks 2 static context positions host-side with this exact rng.
CTX_IDX = [int(v) for v in np.random.default_rng(seed=0).choice(np.arange(S), size=NB_CTX)]

# Dtype used for the PE matmul operands (featT, weights, hiddenT).
#  float32: 4 cycles/row (2 half-speed passes) — exact baseline.
#  float32r: 1 cycle/row at N>=256 — reduced-mantissa single pass.
#  bfloat16: 1 cycle/row + fast weight load, half the SBUF traffic.
# Accumulation is always fp32 in PSUM; center/ctx stay fp32 until the single
# rounding point at the PSUM->featT copy.
MM_DT = mybir.dt.float32r
# Dtype the span gather lands in SBUF as (the DMA casts in flight when bf16 —
# halves the SBUF-port bytes of the SWDGE stream; measured no win, keep f32).
GATHER_DT = mybir.dt.float32

# Const blob layout (f32 columns): identity [0:128), io8f [128:136),
# io36f [136:172).  Separate int32 blob: rowbase [p, 0] = p*S.
C_ID, C_IO8, C_IO36, C_NF = 0, 128, 136, 172


def _build():
    nc = bacc.Bacc(
        "TRN2",
        target_bir_lowering=False,
        debug=False,
        enable_asserts=False,
        num_devices=NCORES,
    )
    emb = nc.dram_tensor("emb", [BC * S, H], F32, kind="ExternalInput").ap()
    pos = nc.dram_tensor("pos", [BC, 2], I32, kind="ExternalInput").ap()
    cat = nc.dram_tensor("cat", [BC, 1], I32, kind="ExternalInput").ap()
    wbT = nc.dram_tensor("wbT", [F3H, INNER], F32, kind="ExternalInput").ap()
    bb = nc.dram_tensor("bb", [INNER], F32, kind="ExternalInput").ap()
    wexpT = nc.dram_tensor("wexpT", [INNER + 1, NE], F32, kind="ExternalInput").ap()
    cstf = nc.dram_tensor("cstf", [128, C_NF], F32, kind="ExternalInput").ap()
    onesd = nc.dram_tensor("onesd", [1, 256], F32, kind="ExternalInput").ap()
    # Host-computed gather row indices, col h*3+ci: chunk start rows, with the
    # skip marker (BIG, beyond the bounds check) baked in for chunks 1/2 when
    # the span doesn't reach them.
    gidx = nc.dram_tensor("gidx", [128, 6], I32, kind="ExternalInput").ap()
    out = nc.dram_tensor("out", [BC, NB_LABELS], F32, kind="ExternalOutput").ap()

    emb3d = emb.rearrange("(b s) h -> b s h", s=S)
    TD = MM_DT
    GDT = GATHER_DT

    def asTD(dram_ap):
        # f32 DRAM bits reinterpreted as the PE dtype (f32r shares the layout).
        return dram_ap.bitcast(TD) if TD == mybir.dt.float32r else dram_ap

    with tile.TileContext(nc) as tc, ExitStack() as ctx:
        pool = ctx.enter_context(tc.tile_pool(name="main", bufs=1))
        gpool = ctx.enter_context(tc.tile_pool(name="gp", bufs=2))
        spool = ctx.enter_context(tc.tile_pool(name="small", bufs=2))
        pst = ctx.enter_context(tc.tile_pool(name="pst", bufs=2, space="PSUM"))
        psh = ctx.enter_context(tc.tile_pool(name="psh", bufs=2, space="PSUM"))
        ps36p = ctx.enter_context(tc.tile_pool(name="ps36p", bufs=2, space="PSUM"))

        # --- phase 0: tiny front-of-queue loads the gather depends on ---
        gidx_t = pool.tile([128, 6], I32)
        nc.sync.dma_start(gidx_t[:], gidx[:, :])
        pos_t = pool.tile([128, 4], I32)  # [p, h*2 + j] = pos[h*128+p, j]
        nc.sync.dma_start(pos_t[:].rearrange("p (h j) -> p h j", j=2),
                          pos.rearrange("(h p) j -> p h j", p=128))
        cstf_t = pool.tile([128, C_NF], F32)
        nc.sync.dma_start(cstf_t[:], cstf[:, :])
        cat_t = pool.tile([128, 2], I32)  # [p, h] = cat[h*128+p]
        nc.sync.dma_start(cat_t[:].rearrange("p (h j) -> p h j", j=1),
                          cat.rearrange("(h p) j -> p h j", p=128))

        io8f = cstf_t[:, C_IO8:C_IO8 + SPAN]
        io36f = cstf_t[:, C_IO36:C_IO36 + NE]
        # Identity for PE transposes, in the PE dtype (separate tile so the
        # fp32r verifier sees a rounded producer).
        id_t = pool.tile([128, 128], TD)
        nc.sync.dma_start(id_t[:], asTD(cstf[:, C_ID:C_ID + 128]))
        identity = id_t[:]

        # Pre-zero the conditional gather chunks first thing on DVE: skipped
        # samples keep zeros, and the zero span weights keep them out of the
        # mean. Must land before the conditional gathers start writing.
        gz = []
        for ci in (1, 2):
            for h in range(2):
                g = gpool.tile([128, 2 * H], GDT, tag=f"g{h}{ci}", bufs=1)
                nc.vector.memset(g[:], 0.0)
                gz.append(g)

        # --- phase 1: per-half index chains + gathers, earliest possible ---
        # Row chunks per sample: [0:4), [4:6), [6:8).  Chunks 1/2 are skipped
        # per-sample via the DGE bounds check when the span doesn't reach them
        # (len<=4 / len<=6); their tiles are pre-zeroed so skipped lanes stay 0
        # and the zero weights keep them out of the mean.
        g_chunks = [[None, None, None], [None, None, None]]
        # chunk 0 (always needed) goes out as soon as its indices land.
        # The two halves ride different SWDGE queue rows so each SDMA engine
        # round-robins between two descriptor streams (hides HBM read latency).
        for h in (1, 0):
            g0 = gpool.tile([128, 4 * H], GDT, tag=f"g{h}0", bufs=1)
            nc.gpsimd.indirect_dma_start(
                out=g0[:], out_offset=None, in_=emb,
                in_offset=IndirectOffsetOnAxis(ap=gidx_t[:, 3 * h:3 * h + 1], axis=0),
            )
            g_chunks[h][0] = g0

        # conditional chunks: gather with the bounds check dropping the
        # per-sample skip-marked indices; interleave halves so each half's
        # last chunk lands as early as possible
        for h, ci in ((0, 1), (0, 2), (1, 1), (1, 2)):
                g = gz[(ci - 1) * 2 + h]
                nc.gpsimd.indirect_dma_start(
                    out=g[:], out_offset=None, in_=emb,
                    in_offset=IndirectOffsetOnAxis(
                        ap=gidx_t[:, 3 * h + ci:3 * h + ci + 1], axis=0),
                    bounds_check=BC * S - 1, oob_is_err=False,
                )
                g_chunks[h][ci] = g

        w8_h = []
        for h in range(2):
            # span weights w8[p, i] = (i < len) / len
            len_i = spool.tile([128, 1], I32, tag=f"leni{h}", bufs=1)
            nc.vector.tensor_tensor(out=len_i[:], in0=pos_t[:, 2 * h + 1:2 * h + 2],
                                    in1=pos_t[:, 2 * h:2 * h + 1],
                                    op=mybir.AluOpType.subtract)
            len_f = spool.tile([128, 1], F32, tag=f"lenf{h}", bufs=1)
            nc.vector.tensor_copy(len_f[:], len_i[:])
            rcp = spool.tile([128, 1], F32, tag=f"rcp{h}", bufs=1)
            nc.vector.reciprocal(rcp[:], len_f[:])
            w8 = spool.tile([128, SPAN], F32, tag=f"w8{h}", bufs=1)
            nc.vector.tensor_scalar(w8[:], io8f, len_f[:, :1], rcp[:, :1],
                                    op0=mybir.AluOpType.is_lt,
                                    op1=mybir.AluOpType.mult)
            w8_h.append(w8)

        # --- phase 2: context rows + replicated weights (overlap gathers) ---
        ctxs = []
        for h in range(2):
            b0 = h * 128
            ctx0 = gpool.tile([128, H], TD, tag=f"ctx0{h}", bufs=1)
            nc.sync.dma_start(ctx0[:], asTD(emb3d[b0:b0 + 128, CTX_IDX[0], :]))
            ctx1 = gpool.tile([128, H], TD, tag=f"ctx1{h}", bufs=1)
            nc.sync.dma_start(ctx1[:], asTD(emb3d[b0:b0 + 128, CTX_IDX[1], :]))
            ctxs.append((ctx0, ctx1))

        # wbT is shipped pre-laid-out: wbT_host[p, c*INNER+m] = W_base[m, c*128+p].
        # Split into 6 medium DMAs so the packets interleave gently with the
        # concurrent indirect gathers.
        wbT_t = pool.tile([128, KC * INNER], TD)
        wbT_c = wbT.rearrange("(p x) m -> p (x m)", p=128)
        step = KC * INNER // 6
        # ctx-chunk weights (cols 6*INNER..) are needed mid-gather by phase 3b;
        # center-chunk weights (cols 0..6*INNER) aren't needed until phase 4.
        for j in (2, 3, 4, 5, 0, 1):
            sl = slice(j * step, (j + 1) * step)
            if TD == mybir.dt.bfloat16:
                nc.gpsimd.dma_start(wbT_t[:, sl], wbT_c[:, sl])
            else:
                nc.sync.dma_start(wbT_t[:, sl], asTD(wbT_c[:, sl]))
        bb_t = pool.tile([128, 2], F32)  # bb_t[p, t] = b_base[t*128 + p]
        nc.sync.dma_start(bb_t[:], bb.rearrange("(t p) -> p t", p=128))
        wexpA = pool.tile([128, NE], TD)
        wexpB = pool.tile([128, NE], TD)
        wexpC = pool.tile([1, NE], TD)
        if TD == mybir.dt.bfloat16:
            nc.gpsimd.dma_start(wexpA[:], wexpT[0:128, :])
            nc.gpsimd.dma_start(wexpB[:], wexpT[128:256, :])
            nc.gpsimd.dma_start(wexpC[:], wexpT[256:257, :])
        else:
            nc.sync.dma_start(wexpA[:], asTD(wexpT[0:128, :]))
            nc.sync.dma_start(wexpB[:], asTD(wexpT[128:256, :]))
            nc.sync.dma_start(wexpC[:], asTD(wexpT[256:257, :]))
        ones1 = pool.tile([1, 256], TD)
        if TD == mybir.dt.float32r:
            nc.sync.dma_start(ones1[:], asTD(onesd[:, :]))
        else:
            nc.vector.memset(ones1[:], 1.0)

        # --- phase 3a: ctx transposes + copies (their data lands early) ---
        featT = pool.tile([128, KC * 256], TD)
        featT3 = featT[:].rearrange("p (si rest) -> p si rest", si=3)
        for h in range(2):
            ctx0, ctx1 = ctxs[h]
            for c in range(HC):
                tpc = pst.tile([128, 2 * 128], TD, tag="tpc")
                for si, src in enumerate((ctx0, ctx1)):
                    nc.tensor.transpose(tpc[:, si * 128:(si + 1) * 128],
                                        src[:, c * 128:(c + 1) * 128], identity)
                col = c * 256 + h * 128
                nc.scalar.copy(featT3[:, 1:3, col:col + 128],
                               tpc[:].rearrange("p (si x) -> p si x", si=2))

        # --- phase 3b: ctx part of the base linear runs during the gather ---
        hiddenT = pool.tile([128, 2 * 256], TD)
        accs = [psh.tile([128, 256], F32, tag=f"acc{mt}", bufs=1, name=f"acc{mt}")
                for mt in range(2)]
        for c in range(HC, KC):
            for mt in range(2):
                nc.tensor.matmul(
                    accs[mt][:],
                    lhsT=wbT_t[:, c * INNER + mt * 128: c * INNER + (mt + 1) * 128],
                    rhs=featT[:, c * 256:(c + 1) * 256],
                    start=(c == HC), stop=False,
                )

        # --- phase 3c: masked mean + center transposes ---
        # h1's chunk 0 lands first in the stream, so its accA chain is emitted
        # first; h0's full stream completes mid-gather, so its accB/transpose
        # pipeline goes next, and only h1's accB tail trails the last chunk.
        accA_h = {}
        for h in (1, 0):
            w8 = w8_h[h]
            accA = gpool.tile([128, H], F32, tag=f"accA{h}", bufs=1,
                              name=f"accA{h}")
            nc.vector.tensor_scalar(accA[:], g_chunks[h][0][:, 0:H], w8[:, 0:1],
                                    None, op0=mybir.AluOpType.mult)
            for i in range(1, 4):
                off = i * H
                nc.vector.scalar_tensor_tensor(
                    out=accA[:], in0=g_chunks[h][0][:, off:off + H],
                    scalar=w8[:, i:i + 1], in1=accA[:],
                    op0=mybir.AluOpType.mult, op1=mybir.AluOpType.add)
            accA_h[h] = accA

        catf_h = [None, None]
        for h in range(2):
            w8 = w8_h[h]
            # accB: rows 4-7 (conditional chunks land last)
            accB = gpool.tile([128, H], F32, tag=f"accB{h}", bufs=1)
            nc.vector.tensor_scalar(accB[:], g_chunks[h][1][:, 0:H], w8[:, 4:5],
                                    None, op0=mybir.AluOpType.mult)
            for i, (ci, off) in enumerate([(1, H), (2, 0), (2, H)], start=5):
                nc.vector.scalar_tensor_tensor(
                    out=accB[:], in0=g_chunks[h][ci][:, off:off + H],
                    scalar=w8[:, i:i + 1], in1=accB[:],
                    op0=mybir.AluOpType.mult, op1=mybir.AluOpType.add)
            center = gpool.tile([128, H], TD, tag=f"center{h}", bufs=1)
            nc.vector.tensor_tensor(out=center[:], in0=accA_h[h][:], in1=accB[:],
                                    op=mybir.AluOpType.add)

            # center transposes; one ACT copy per h-chunk keeps DVE free
            for c in range(HC):
                tp = pst.tile([128, 128], TD, tag="tp")
                nc.tensor.transpose(tp[:], center[:, c * 128:(c + 1) * 128],
                                    identity)
                col = c * 256 + h * 128
                nc.scalar.copy(featT3[:, 0:1, col:col + 128],
                               tp[:].rearrange("p (si x) -> p si x", si=1))

            catf = spool.tile([128, 1], F32, tag=f"catf{h}", bufs=1)
            nc.vector.tensor_copy(catf[:], cat_t[:, h:h + 1])
            catf_h[h] = catf

        # --- phase 4: center chunks close the accumulation; bias+relu fused ---
        for c in range(HC):
            for mt in range(2):
                nc.tensor.matmul(
                    accs[mt][:],
                    lhsT=wbT_t[:, c * INNER + mt * 128: c * INNER + (mt + 1) * 128],
                    rhs=featT[:, c * 256:(c + 1) * 256],
                    start=False, stop=(c == HC - 1),
                )
        for mt in range(2):
            nc.scalar.activation(hiddenT[:, mt * 256:(mt + 1) * 256], accs[mt][:],
                                 mybir.ActivationFunctionType.Relu,
                                 bias=bb_t[:, mt:mt + 1], scale=1.0)

        # --- phase 5: expert heads + per-sample selection ---
        out3 = pool.tile([128, 2 * NB_LABELS], F32)  # [p, h*3 + n]
        for h in range(2):
            b0 = h * 128
            mask36 = spool.tile([128, NE], F32, tag="mask36")
            nc.vector.tensor_scalar(mask36[:], io36f, catf_h[h][:, :1], None,
                                    op0=mybir.AluOpType.is_equal)
            ps36 = ps36p.tile([128, NE], F32, tag="ps36")
            nc.tensor.matmul(ps36[:], lhsT=hiddenT[:, b0:b0 + 128],
                             rhs=wexpA[:], start=True, stop=False)
            nc.tensor.matmul(ps36[:], lhsT=hiddenT[:, 256 + b0:256 + b0 + 128],
                             rhs=wexpB[:], start=False, stop=False)
            nc.tensor.matmul(ps36[:], lhsT=ones1[:, b0:b0 + 128],
                             rhs=wexpC[:], start=False, stop=True)

            prod = spool.tile([128, NE], F32, tag="prod")
            nc.vector.tensor_tensor(out=prod[:], in0=ps36[:], in1=mask36[:],
                                    op=mybir.AluOpType.mult)
            nc.vector.tensor_reduce(
                out=out3[:, h * NB_LABELS:(h + 1) * NB_LABELS],
                in_=prod[:].rearrange("p (e n) -> p n e", n=NB_LABELS),
                axis=mybir.AxisListType.X, op=mybir.AluOpType.add)
        nc.sync.dma_start(out.rearrange("(h p) n -> p h n", p=128),
                          out3[:].rearrange("p (h n) -> p h n", n=NB_LABELS))

    nc.compile()
    return nc


_NC = None


def _get_nc():
    global _NC
    if _NC is None:
        _NC = _build()
    return _NC


def _const_blobs():
    cstf = np.zeros((128, C_NF), dtype=np.float32)
    cstf[:, C_ID:C_ID + 128] = np.eye(128, dtype=np.float32)
    cstf[:, C_IO8:C_IO8 + SPAN] = np.arange(SPAN, dtype=np.float32)[None, :]
    cstf[:, C_IO36:C_IO36 + NE] = np.repeat(
        np.arange(NB_EXPERTS, dtype=np.float32), NB_LABELS)[None, :]
    return cstf


def _prep_inputs(embeddings, position_indexes, categories, W_base, b_base,
                 W_experts, b_experts):
    emb = np.ascontiguousarray(np.asarray(embeddings, dtype=np.float32)).reshape(
        NCORES, BC * S, H)
    pos = np.ascontiguousarray(np.asarray(position_indexes).astype(np.int32)).reshape(
        NCORES, BC, 2)
    cat = np.ascontiguousarray(np.asarray(categories).astype(np.int32)).reshape(
        NCORES, BC, 1)
    # wbT_host[p, c*INNER+m] = W_base[m, c*128+p]; shipped as [3H, INNER] rows
    # grouped so the device DMA is a single contiguous [128, 18*256] copy.
    wb = np.asarray(W_base, dtype=np.float32)  # [INNER, 3H]
    wbT = np.ascontiguousarray(
        wb.T.reshape(KC, 128, INNER).transpose(1, 0, 2).reshape(128, KC * INNER)
    ).reshape(F3H, INNER)  # same bytes, declared [3H, INNER] for the DRAM tensor
    bb = np.ascontiguousarray(np.asarray(b_base, dtype=np.float32))
    we = np.asarray(W_experts, dtype=np.float32)  # [12, 3, INNER]
    be = np.asarray(b_experts, dtype=np.float32)  # [12, 3]
    wexpT = np.concatenate(
        [we.transpose(2, 0, 1).reshape(INNER, NE), be.reshape(1, NE)], axis=0)
    wexpT = np.ascontiguousarray(wexpT)  # [INNER+1, 36]
    cstf = _const_blobs()

    # Per-core gather row indices [128, 6]: col h*3+ci holds the first row of
    # span chunk ci ([0:4), [4:6), [6:8)) for sample h*128+p, or BIG when the
    # span doesn't reach that chunk (dropped by the DGE bounds check).
    BIG = 100000
    starts = pos[:, :, 0].astype(np.int64)                  # [NCORES, BC]
    lens = (pos[:, :, 1] - pos[:, :, 0]).astype(np.int64)
    base = np.arange(BC, dtype=np.int64) * S
    i0 = base[None, :] + starts
    c1 = np.where(lens > 4, i0 + 4, BIG)
    c2 = np.where(lens > 6, i0 + 6, BIG)
    gidx = np.stack([i0, c1, c2], axis=-1).reshape(NCORES, 2, 128, 3)
    gidx = np.ascontiguousarray(
        gidx.transpose(0, 2, 1, 3).reshape(NCORES, 128, 6).astype(np.int32))

    return [
        {"emb": emb[i], "pos": pos[i], "cat": cat[i], "wbT": wbT, "bb": bb,
         "wexpT": wexpT, "cstf": cstf, "gidx": gidx[i],
         "onesd": np.ones((1, 256), dtype=np.float32)}
        for i in range(NCORES)
    ]


def _run(in_maps, **kw):
    nc = _get_nc()
    return run_bass_kernel_spmd(nc, in_maps, core_ids=list(range(NCORES)), **kw)


def kernel(embeddings, position_indexes, categories, W_base, b_base, W_experts,
           b_experts):
    in_maps = _prep_inputs(embeddings, position_indexes, categories, W_base,
                           b_base, W_experts, b_experts)
    res = _run(in_maps)
    return np.concatenate([r["out"] for r in res.results], axis=0)



# revision 24
# speedup vs baseline: 1.1045x; 1.1045x over previous
"""Trainium2 Bass kernel for nn_BertClassifier_77309411685 (V2).

Data-parallel over 8 NeuronCores: each core handles 256 samples; the small
base linear and 12 expert heads are replicated.

V2 strategy (vs the f32 baseline):
  * fp16 end-to-end: embeddings / weights are host-cast to fp16, halving all
    HBM traffic.  PSUM accumulation stays fp32.
  * span gather: per half (128 samples), TWO indirect DMAs into one
    [128, 4*H] tile.  Op LO gathers span rows 0..3 (rows past the span
    length redirect to a zeros row appended to emb, so every lane is real
    data).  Op HI gathers rows 4..7 with skip markers dropped by the DGE
    bounds check and compute_op=add, accumulating slot-wise onto LO's tile.
    Expected traffic 5.25 rows/sample instead of 8; no memsets, no stale
    lanes.
  * masked mean: slot sums are 3 fp16 DVE adds per half; the 1/len scale is
    folded into the PE transpose by using host-built diag(1/len) matrices in
    place of the identity.
  * the 2 static context rows are host-sliced and shipped pre-transposed in
    featT layout, DMA'd straight into the featT tile (no PE transposes).
  * base linear: ctx k-chunks run mid-gather; center k-chunks are split per
    half so h0's matmuls overlap h1's gather/mean; bias+relu fused in the
    PSUM->SBUF activation, per (m-tile, half).
  * expert heads: all 12 experts at once with bias folded via a ones row;
    per-sample selection by is_equal mask + strided reduce (as baseline).
"""

import numpy as np
from contextlib import ExitStack

import concourse.bass as bass
import concourse.tile as tile
from concourse import bacc, mybir
from concourse.bass import IndirectOffsetOnAxis
from concourse.bass_utils import run_bass_kernel_spmd

F32 = mybir.dt.float32
F16 = mybir.dt.float16
I32 = mybir.dt.int32

B, S, H = 2048, 256, 768
INNER, NB_CTX, NB_EXPERTS, NB_LABELS = 256, 2, 12, 3
NCORES = 8
BC = B // NCORES             # 256 samples per core
F3H = (NB_CTX + 1) * H       # 2304
KC = F3H // 128              # 18 contraction chunks
HC = H // 128                # 6 chunks per feature block
NE = NB_EXPERTS * NB_LABELS  # 36
SPAN = 8
# The HW DGE misreads multi-index offset APs (wrong rows for large index
# values, corrupted neighbours, broken accumulate slots — all HW-measured).
# Only single-index-per-partition offset APs are reliable, so each gather op
# moves one CONTIGUOUS 4-row block per sample (one 6KB descriptor).  Rows
# past the span inside a block are real data killed by a 0/1 mask on DVE;
# the rows-4..7 block is redirected wholesale to appended zero rows when the
# span doesn't reach it.
HROWS = 128 * S              # 32768 real rows per half-batch tensor
ZROW = HROWS                 # first of 8 zero rows appended
NROWS = HROWS + SPAN

# The reference picks 2 static context positions host-side with this exact rng.
CTX_IDX = [int(v) for v in np.random.default_rng(seed=0).choice(np.arange(S), size=NB_CTX)]




def _build():
    nc = bacc.Bacc(
        "TRN2",
        target_bir_lowering=False,
        debug=False,
        enable_asserts=False,
        num_devices=NCORES,
    )
    embs = [nc.dram_tensor(f"emb{h}", [NROWS, H], F16, kind="ExternalInput").ap()
            for h in range(2)]
    gidx = nc.dram_tensor("gidx", [128, 4], I32, kind="ExternalInput").ap()
    wbT = nc.dram_tensor("wbT", [F3H, INNER], F16, kind="ExternalInput").ap()
    ctxT = nc.dram_tensor("ctxT", [128, NB_CTX * HC * 256], F16, kind="ExternalInput").ap()
    dcst = nc.dram_tensor("dcst", [128, 2 * 128], F16, kind="ExternalInput").ap()
    # cst32: io36 [0:36) + categories-as-float [36:38) + span masks [38:54)
    cst32 = nc.dram_tensor("cst32", [128, NE + 2 + 16], F32, kind="ExternalInput").ap()
    bb = nc.dram_tensor("bb", [INNER], F32, kind="ExternalInput").ap()
    wexpT = nc.dram_tensor("wexpT", [INNER + 1, NE], F16, kind="ExternalInput").ap()
    onesd = nc.dram_tensor("onesd", [1, 256], F16, kind="ExternalInput").ap()
    out = nc.dram_tensor("out", [BC, NB_LABELS], F32, kind="ExternalOutput").ap()

    with tile.TileContext(nc) as tc, ExitStack() as ctx:
        pool = ctx.enter_context(tc.tile_pool(name="main", bufs=1))
        gpool = ctx.enter_context(tc.tile_pool(name="gp", bufs=2))
        spool = ctx.enter_context(tc.tile_pool(name="small", bufs=2))
        pst = ctx.enter_context(tc.tile_pool(name="pst", bufs=2, space="PSUM"))
        psh = ctx.enter_context(tc.tile_pool(name="psh", bufs=2, space="PSUM"))
        ps36p = ctx.enter_context(tc.tile_pool(name="ps36p", bufs=2, space="PSUM"))

        # --- phase 0: tiny front-of-queue loads the gathers depend on ---
        gidx_t = pool.tile([128, 4], I32)
        nc.sync.dma_start(gidx_t[:], gidx[:, :])

        # --- phase 1: span gathers, earliest possible ---
        # Per half: block A = rows start..start+3 (one contiguous 4-row
        # descriptor per sample), block D = rows start+4..start+7, redirected
        # to the appended zero rows when len <= 4.  Single-index offset APs
        # only (multi-index APs are unreliable on HW).
        gA_h, gD_h = [], []
        for h in range(2):
            gA = gpool.tile([128, 4 * H], F16, tag=f"gA{h}", bufs=1)
            nc.gpsimd.indirect_dma_start(
                out=gA[:], out_offset=None, in_=embs[h],
                in_offset=IndirectOffsetOnAxis(ap=gidx_t[:, 2 * h:2 * h + 1], axis=0),
            )
            gA_h.append(gA)
        for h in range(2):
            gD = gpool.tile([128, 4 * H], F16, tag=f"gD{h}", bufs=1)
            nc.gpsimd.indirect_dma_start(
                out=gD[:], out_offset=None, in_=embs[h],
                in_offset=IndirectOffsetOnAxis(ap=gidx_t[:, 2 * h + 1:2 * h + 2], axis=0),
            )
            gD_h.append(gD)

        # --- phase 2: small consts first (cheap, needed mid-kernel), then the
        # context block + replicated weights (overlap gathers) ---
        dcst_t = pool.tile([128, 2 * 128], F16)
        nc.sync.dma_start(dcst_t[:], dcst[:, :])
        cst32_t = pool.tile([128, NE + 2 + 16], F32)
        nc.sync.dma_start(cst32_t[:], cst32[:, :])
        io36f = cst32_t[:, 0:NE]
        catf = cst32_t[:, NE:NE + 2]
        bb_t = pool.tile([128, 2], F32)  # bb_t[p, t] = b_base[t*128 + p]
        nc.sync.dma_start(bb_t[:], bb.rearrange("(t p) -> p t", p=128))
        wexpA = pool.tile([128, NE], F16)
        wexpB = pool.tile([128, NE], F16)
        wexpC = pool.tile([1, NE], F16)
        nc.sync.dma_start(wexpA[:], wexpT[0:128, :])
        nc.sync.dma_start(wexpB[:], wexpT[128:256, :])
        nc.sync.dma_start(wexpC[:], wexpT[256:257, :])
        ones1 = pool.tile([1, 256], F16)
        nc.sync.dma_start(ones1[:], onesd[:, :])

        featT = pool.tile([128, KC * 256], F16)
        # static ctx rows arrive pre-transposed in exact featT layout
        nc.sync.dma_start(featT[:, HC * 256:KC * 256], ctxT[:, :])
        wbT_t = pool.tile([128, KC * INNER], F16)
        wbT_c = wbT.rearrange("(p x) m -> p (x m)", p=128)
        # ctx-chunk weights are needed first (phase 3b), center-chunk weights last
        nc.sync.dma_start(wbT_t[:, HC * INNER:], wbT_c[:, HC * INNER:])
        nc.sync.dma_start(wbT_t[:, :HC * INNER], wbT_c[:, :HC * INNER])

        # PE warm-up: the HAM clock gate releases after ~3.4us of sustained
        # activity; a burst of throwaway matmuls on the already-loaded dcst
        # tile warms the array before the real matmuls arrive.
        warm = pst.tile([128, 256], F32, tag="warm", bufs=1)
        for w in range(8):
            nc.tensor.matmul(warm[:], lhsT=dcst_t[:, 0:128], rhs=dcst_t[:, 0:256],
                             start=(w == 0), stop=(w == 7))

        # --- phase 3b/3c/4 interleaved per half ---
        # All base-linear matmuls are N=128, grouped per (m-tile, half): the
        # ctx chunks open each accumulation group (overlapping the gathers),
        # the center chunks close it.
        accs = [psh.tile([128, 256], F32, tag=f"acc{mt}", bufs=1, name=f"acc{mt}")
                for mt in range(2)]

        def ctx_mms(h):
            for c in range(HC, KC):
                for mt in range(2):
                    nc.tensor.matmul(
                        accs[mt][:, h * 128:(h + 1) * 128],
                        lhsT=wbT_t[:, c * INNER + mt * 128: c * INNER + (mt + 1) * 128],
                        rhs=featT[:, c * 256 + h * 128: c * 256 + h * 128 + 128],
                        start=(c == HC), stop=False,
                    )

        featT_pairs = featT[:].rearrange("p (c x) -> p c x", x=256)
        hiddenT = pool.tile([128, 2 * 256], F16)
        m8 = cst32_t[:, NE + 2:NE + 2 + 16]
        for h in range(2):
            ctx_mms(h)
            gA, gD = gA_h[h], gD_h[h]
            # masked sum over the 8 slots: rows past the span are real junk
            # (block A) or zeros (block D) — the 0/1 masks kill both.  Two
            # independent chains, combined at the end (shorter dep chain).
            mc = lambda j: m8[:, 8 * h + j:8 * h + j + 1]
            ca = gpool.tile([128, H], F16, tag=f"ca{h}", bufs=1)
            nc.vector.tensor_scalar(ca[:], gA[:, 0:H], mc(0), None,
                                    op0=mybir.AluOpType.mult)
            for j in range(1, 4):
                nc.vector.scalar_tensor_tensor(
                    out=ca[:], in0=gA[:, j * H:(j + 1) * H], scalar=mc(j),
                    in1=ca[:], op0=mybir.AluOpType.mult, op1=mybir.AluOpType.add)
            cd = gpool.tile([128, H], F16, tag=f"cd{h}", bufs=1)
            nc.vector.tensor_scalar(cd[:], gD[:, 0:H], mc(4), None,
                                    op0=mybir.AluOpType.mult)
            for j in range(5, 8):
                nc.vector.scalar_tensor_tensor(
                    out=cd[:], in0=gD[:, (j - 4) * H:(j - 3) * H], scalar=mc(j),
                    in1=cd[:], op0=mybir.AluOpType.mult, op1=mybir.AluOpType.add)
            ct = gpool.tile([128, H], F16, tag=f"ct{h}", bufs=1)
            nc.vector.tensor_tensor(out=ct[:], in0=ca[:], in1=cd[:],
                                    op=mybir.AluOpType.add)

            # center transposes scaled by diag(1/len): centerT = ct.T @ diag.
            # A regular matmul (not PE transpose mode, which requires a pure
            # permutation matrix and would drop the scale).
            diag = dcst_t[:, h * 128:(h + 1) * 128]
            for cp in range(HC // 2):
                tpc = pst.tile([128, 256], F32, tag="tpc")
                for k in range(2):
                    c = 2 * cp + k
                    nc.tensor.matmul(tpc[:, k * 128:(k + 1) * 128],
                                     lhsT=ct[:, c * 128:(c + 1) * 128],
                                     rhs=diag, start=True, stop=True)
                nc.scalar.copy(
                    featT_pairs[:, 2 * cp:2 * cp + 2, h * 128:(h + 1) * 128],
                    tpc[:].rearrange("p (k x) -> p k x", k=2))

            # --- phase 4 (per half): center chunks close the accumulation ---
            for c in range(HC):
                for mt in range(2):
                    nc.tensor.matmul(
                        accs[mt][:, h * 128:(h + 1) * 128],
                        lhsT=wbT_t[:, c * INNER + mt * 128: c * INNER + (mt + 1) * 128],
                        rhs=featT[:, c * 256 + h * 128: c * 256 + h * 128 + 128],
                        start=False, stop=(c == HC - 1),
                    )
            for mt in range(2):
                nc.scalar.activation(
                    hiddenT[:, mt * 256 + h * 128: mt * 256 + h * 128 + 128],
                    accs[mt][:, h * 128:(h + 1) * 128],
                    mybir.ActivationFunctionType.Relu,
                    bias=bb_t[:, mt:mt + 1], scale=1.0)

        # --- phase 5: expert heads + per-sample selection ---
        out3 = pool.tile([128, 2 * NB_LABELS], F32)  # [p, h*3 + n]
        for h in range(2):
            b0 = h * 128
            mask36 = spool.tile([128, NE], F32, tag="mask36")
            nc.vector.tensor_scalar(mask36[:], io36f, catf[:, h:h + 1], None,
                                    op0=mybir.AluOpType.is_equal)
            ps36 = ps36p.tile([128, NE], F32, tag="ps36")
            nc.tensor.matmul(ps36[:], lhsT=hiddenT[:, b0:b0 + 128],
                             rhs=wexpA[:], start=True, stop=False)
            nc.tensor.matmul(ps36[:], lhsT=hiddenT[:, 256 + b0:256 + b0 + 128],
                             rhs=wexpB[:], start=False, stop=False)
            nc.tensor.matmul(ps36[:], lhsT=ones1[:, b0:b0 + 128],
                             rhs=wexpC[:], start=False, stop=True)

            prod = spool.tile([128, NE], F32, tag="prod")
            nc.vector.tensor_tensor(out=prod[:], in0=ps36[:], in1=mask36[:],
                                    op=mybir.AluOpType.mult)
            nc.vector.tensor_reduce(
                out=out3[:, h * NB_LABELS:(h + 1) * NB_LABELS],
                in_=prod[:].rearrange("p (e n) -> p n e", n=NB_LABELS),
                axis=mybir.AxisListType.X, op=mybir.AluOpType.add)
        nc.sync.dma_start(out.rearrange("(h p) n -> p h n", p=128),
                          out3[:].rearrange("p (h n) -> p h n", n=NB_LABELS))

    nc.compile()
    return nc


_NC = None


def _get_nc():
    global _NC
    if _NC is None:
        _NC = _build()
    return _NC


def _prep_inputs(embeddings, position_indexes, categories, W_base, b_base,
                 W_experts, b_experts):
    emb32 = np.asarray(embeddings)
    emb16 = emb32.astype(np.float16).reshape(NCORES, 2, HROWS, H)
    zpad = np.zeros((SPAN, H), dtype=np.float16)

    pos = np.asarray(position_indexes).astype(np.int64).reshape(NCORES, BC, 2)
    cat = np.asarray(categories).astype(np.int64).reshape(NCORES, BC)

    # gather block-start indices [128, 4]: col 2h+0 = row of span start for
    # sample h*128+p in that half's emb tensor (block A, rows start..start+3);
    # col 2h+1 = start+4 when the span reaches rows 4..7, else the zero rows.
    starts = pos[:, :, 0].reshape(NCORES, 2, 128)
    lens = (pos[:, :, 1] - pos[:, :, 0]).reshape(NCORES, 2, 128)
    base = np.arange(128, dtype=np.int64) * S
    rowA = base[None, None, :] + starts                       # [NC, 2, 128]
    rowD = np.where(lens > 4, rowA + 4, ZROW)
    gidx = np.stack([rowA, rowD], axis=-1)                    # [NC, 2, 128, 2]
    gidx = gidx.transpose(0, 2, 1, 3).reshape(NCORES, 128, 4).astype(np.int32)

    # dcst: diag(1/len) per half [0:256) + 0/1 span masks [256:272)
    rcp = 1.0 / lens.astype(np.float32)  # [NC, 2, 128]
    eye = np.eye(128, dtype=np.float32)
    diags = (eye[None, None] * rcp[:, :, :, None]).transpose(0, 2, 1, 3).reshape(
        NCORES, 128, 256)
    dcst = diags.astype(np.float16)

    # base linear: wbT_host[p, c*INNER+m] = W_base[m, c*128+p]
    wb = np.asarray(W_base, dtype=np.float32)  # [INNER, 3H]
    wbT = np.ascontiguousarray(
        wb.T.reshape(KC, 128, INNER).transpose(1, 0, 2).reshape(128, KC * INNER)
    ).astype(np.float16).reshape(F3H, INNER)

    bbias = np.ascontiguousarray(np.asarray(b_base, dtype=np.float32))

    we = np.asarray(W_experts, dtype=np.float32)  # [12, 3, INNER]
    be = np.asarray(b_experts, dtype=np.float32)  # [12, 3]
    wexpT = np.ascontiguousarray(np.concatenate(
        [we.transpose(2, 0, 1).reshape(INNER, NE), be.reshape(1, NE)],
        axis=0)).astype(np.float16)

    # static context rows, pre-transposed into featT layout:
    # ctxT[p, (which*6+cc)*256 + h*128 + sl] = emb[s(h,sl), CTX_IDX[which], cc*128+p]
    emb3 = emb16.reshape(NCORES, BC, S, H)
    blocks = []
    for which in range(NB_CTX):
        blk = emb3[:, :, CTX_IDX[which], :]                     # [NC, 256, 768]
        arr = blk.reshape(NCORES, 2, 128, HC, 128).transpose(0, 4, 3, 1, 2)
        blocks.append(arr.reshape(NCORES, 128, HC * 256))
    ctxT = np.ascontiguousarray(np.concatenate(blocks, axis=2))  # [NC, 128, 12*256]

    # io36 + categories-as-float + span masks (f32)
    cst32 = np.zeros((NCORES, 128, NE + 2 + 16), dtype=np.float32)
    cst32[:, :, :NE] = np.repeat(np.arange(NB_EXPERTS, dtype=np.float32),
                                 NB_LABELS)[None, None, :]
    cst32[:, :, NE:NE + 2] = cat.reshape(NCORES, 2, 128).transpose(0, 2, 1)
    j = np.arange(SPAN, dtype=np.int64)
    m8 = (j[None, None, None, :] < lens[..., None]).astype(np.float32)
    cst32[:, :, NE + 2:] = m8.transpose(0, 2, 1, 3).reshape(NCORES, 128, 16)

    onesd = np.ones((1, 256), dtype=np.float16)

    return [
        {"emb0": np.ascontiguousarray(np.concatenate([emb16[i, 0], zpad], axis=0)),
         "emb1": np.ascontiguousarray(np.concatenate([emb16[i, 1], zpad], axis=0)),
         "gidx": np.ascontiguousarray(gidx[i]),
         "wbT": wbT, "ctxT": ctxT[i], "dcst": np.ascontiguousarray(dcst[i]),
         "cst32": np.ascontiguousarray(cst32[i]), "bb": bbias,
         "wexpT": wexpT, "onesd": onesd}
        for i in range(NCORES)
    ]


def _run(in_maps, **kw):
    nc = _get_nc()
    return run_bass_kernel_spmd(nc, in_maps, core_ids=list(range(NCORES)), **kw)


def kernel(embeddings, position_indexes, categories, W_base, b_base, W_experts,
           b_experts):
    in_maps = _prep_inputs(embeddings, position_indexes, categories, W_base,
                           b_base, W_experts, b_experts)
    res = _run(in_maps)
    return np.concatenate([r["out"] for r in res.results], axis=0)


# revision 27
# speedup vs baseline: 1.2065x; 1.0923x over previous
"""Trainium2 Bass kernel for nn_BertClassifier_77309411685 (V7).

Data-parallel over 8 NeuronCores: each core handles 256 samples; the small
base linear and 12 expert heads are replicated.

Strategy:
  * fp16 end-to-end (host-cast); PSUM accumulation stays fp32.
  * span gather: ONE indirect DMA per half-batch of 128 samples — 8
    contiguous rows from `start` as a single 12KB descriptor per sample
    (spans always fit inside the sample's S rows, so no bounds handling).
    Rows past the span are real data, killed by masks later.  Single-index
    offset APs only: the HW DGE misreads multi-index offset APs.
  * masked mean on the PE: per half, 8 accumulating matmuls with stationary
    diag(mask_j/len) (built on DVE from a shipped diag(1/len) and 0/1 span
    masks) against the gathered rows -> center in PSUM, already averaged.
  * the 2 static context rows are host-sliced and shipped pre-transposed in
    featT layout, DMA'd straight into the featT tile.
  * base linear: ctx k-chunks run as soon as their weights land (DMA'd
    before the gathers queue); center k-chunks per half close the PSUM
    accumulation; bias+relu fused in the PSUM->SBUF activation.
  * expert heads: all 12 experts at once with bias folded via a ones row;
    per-sample selection by is_equal mask + strided reduce.
"""

import numpy as np
from contextlib import ExitStack

import concourse.bass as bass
import concourse.tile as tile
from concourse import bacc, mybir
from concourse.bass import IndirectOffsetOnAxis
from concourse.bass_utils import run_bass_kernel_spmd

F32 = mybir.dt.float32
F16 = mybir.dt.float16
I32 = mybir.dt.int32

B, S, H = 2048, 256, 768
INNER, NB_CTX, NB_EXPERTS, NB_LABELS = 256, 2, 12, 3
NCORES = 8
BC = B // NCORES             # 256 samples per core
F3H = (NB_CTX + 1) * H       # 2304
KC = F3H // 128              # 18 contraction chunks
HC = H // 128                # 6 chunks per feature block
NE = NB_EXPERTS * NB_LABELS  # 36
SPAN = 8
HROWS = 128 * S              # rows per half-batch tensor

# The reference picks 2 static context positions host-side with this exact rng.
CTX_IDX = [int(v) for v in np.random.default_rng(seed=0).choice(np.arange(S), size=NB_CTX)]




def _build():
    nc = bacc.Bacc(
        "TRN2",
        target_bir_lowering=False,
        debug=False,
        enable_asserts=False,
        num_devices=NCORES,
    )
    embs = [nc.dram_tensor(f"emb{h}", [HROWS, H], F16, kind="ExternalInput").ap()
            for h in range(2)]
    gidx = nc.dram_tensor("gidx", [128, 2], I32, kind="ExternalInput").ap()
    wbT = nc.dram_tensor("wbT", [F3H, INNER], F16, kind="ExternalInput").ap()
    ctxT = nc.dram_tensor("ctxT", [128, NB_CTX * HC * 256], F16, kind="ExternalInput").ap()
    # dcst: diag(1/len) per half [0:256) + identity [256:384)
    dcst = nc.dram_tensor("dcst", [128, 3 * 128], F16, kind="ExternalInput").ap()
    # cst32: io36 [0:36) + categories-as-float [36:38) + span masks [38:54)
    cst32 = nc.dram_tensor("cst32", [128, NE + 2 + 16], F32, kind="ExternalInput").ap()
    bb = nc.dram_tensor("bb", [INNER], F32, kind="ExternalInput").ap()
    wexpT = nc.dram_tensor("wexpT", [INNER + 1, NE], F16, kind="ExternalInput").ap()
    onesd = nc.dram_tensor("onesd", [1, 256], F16, kind="ExternalInput").ap()
    out = nc.dram_tensor("out", [BC, NB_LABELS], F32, kind="ExternalOutput").ap()

    with tile.TileContext(nc) as tc, ExitStack() as ctx:
        pool = ctx.enter_context(tc.tile_pool(name="main", bufs=1))
        gpool = ctx.enter_context(tc.tile_pool(name="gp", bufs=2))
        spool = ctx.enter_context(tc.tile_pool(name="small", bufs=2))
        pst = ctx.enter_context(tc.tile_pool(name="pst", bufs=2, space="PSUM"))
        psh = ctx.enter_context(tc.tile_pool(name="psh", bufs=1, space="PSUM"))
        ps36p = ctx.enter_context(tc.tile_pool(name="ps36p", bufs=1, space="PSUM"))

        # --- phase 0: tiny front-of-queue loads the gathers depend on ---
        gidx_t = pool.tile([128, 2], I32)
        nc.sync.dma_start(gidx_t[:], gidx[:, :])

        # --- phase 1: span gathers — one op per half, 8 contiguous rows per
        # sample as a single 12KB descriptor (single-index offset AP only:
        # the HW DGE misreads multi-index offset APs) ---
        g_h = []
        for h in range(2):
            g = gpool.tile([128, SPAN * H], F16, tag=f"g{h}", bufs=1)
            nc.gpsimd.indirect_dma_start(
                out=g[:], out_offset=None, in_=embs[h],
                in_offset=IndirectOffsetOnAxis(ap=gidx_t[:, h:h + 1], axis=0),
            )
            g_h.append(g)

        # --- phase 2: small consts first (cheap, needed mid-kernel), then the
        # context block + replicated weights (overlap gathers) ---
        dcst_t = pool.tile([128, 3 * 128], F16)
        nc.sync.dma_start(dcst_t[:], dcst[:, :])
        cst32_t = pool.tile([128, NE + 2 + 16], F32)
        nc.sync.dma_start(cst32_t[:], cst32[:, :])
        io36f = cst32_t[:, 0:NE]
        catf = cst32_t[:, NE:NE + 2]
        bb_t = pool.tile([128, 2], F32)  # bb_t[p, t] = b_base[t*128 + p]
        nc.sync.dma_start(bb_t[:], bb.rearrange("(t p) -> p t", p=128))
        wexpA = pool.tile([128, NE], F16)
        wexpB = pool.tile([128, NE], F16)
        wexpC = pool.tile([1, NE], F16)
        nc.sync.dma_start(wexpA[:], wexpT[0:128, :])
        nc.sync.dma_start(wexpB[:], wexpT[128:256, :])
        nc.sync.dma_start(wexpC[:], wexpT[256:257, :])
        ones1 = pool.tile([1, 256], F16)
        nc.sync.dma_start(ones1[:], onesd[:, :])

        featT = pool.tile([128, KC * 256], F16)
        # static ctx rows arrive pre-transposed in exact featT layout
        nc.sync.dma_start(featT[:, HC * 256:KC * 256], ctxT[:, :])
        wbT_t = pool.tile([128, KC * INNER], F16)
        wbT_c = wbT.rearrange("(p x) m -> p (x m)", p=128)
        # ctx-chunk weights are needed first (phase 3b), center-chunk weights last
        nc.sync.dma_start(wbT_t[:, HC * INNER:], wbT_c[:, HC * INNER:])
        nc.sync.dma_start(wbT_t[:, :HC * INNER], wbT_c[:, :HC * INNER])

        # PE warm-up: the HAM clock gate releases after ~3.4us of sustained
        # activity; a burst of throwaway matmuls on the already-loaded dcst
        # tile warms the array before the real matmuls arrive.
        warm = pst.tile([128, 256], F32, tag="warm", bufs=1)
        for w in range(8):
            nc.tensor.matmul(warm[:], lhsT=dcst_t[:, 0:128], rhs=dcst_t[:, 0:256],
                             start=(w == 0), stop=(w == 7))

        # --- phase 3b/3c/4 interleaved per half ---
        # All base-linear matmuls are N=128, grouped per (m-tile, half): the
        # ctx chunks open each accumulation group (overlapping the gathers),
        # the center chunks close it.
        accs = [psh.tile([128, 256], F32, tag=f"acc{mt}", bufs=1, name=f"acc{mt}")
                for mt in range(2)]

        def ctx_mms(h):
            for c in range(HC, KC):
                for mt in range(2):
                    nc.tensor.matmul(
                        accs[mt][:, h * 128:(h + 1) * 128],
                        lhsT=wbT_t[:, c * INNER + mt * 128: c * INNER + (mt + 1) * 128],
                        rhs=featT[:, c * 256 + h * 128: c * 256 + h * 128 + 128],
                        start=(c == HC), stop=False,
                    )

        # diag(mask_j/len) stationaries for the PE mean: 8 per half, built
        # with cheap packed DVE tensor_scalar ops from diag(1/len) x m8[j].
        m8 = cst32_t[:, NE + 2:NE + 2 + 16]
        dmask = [pool.tile([128, SPAN * 128], F16, name=f"dmask{h}")
                 for h in range(2)]
        for h in range(2):
            diag = dcst_t[:, h * 128:(h + 1) * 128]
            for j in range(SPAN):
                nc.vector.tensor_scalar(
                    dmask[h][:, j * 128:(j + 1) * 128], diag,
                    m8[:, 8 * h + j:8 * h + j + 1], None,
                    op0=mybir.AluOpType.mult)

        featT_pairs = featT[:].rearrange("p (c x) -> p c x", x=256)
        hiddenT = pool.tile([128, 2 * 256], F16)
        identity = dcst_t[:, 256:384]
        for h in range(2):
            ctx_mms(h)
            g = g_h[h]
            # masked mean on the PE: center[q, x] = sum_j m_j[q]/len[q] *
            # g[q, j*H+x], accumulated in PSUM over the 8 slots.  Split at
            # 512 columns (PSUM bank limit).
            psa = pst.tile([128, 512], F32, tag="psa", bufs=1)
            psb = pst.tile([128, 256], F32, tag="psb", bufs=1)
            for j in range(SPAN):
                dm = dmask[h][:, j * 128:(j + 1) * 128]
                nc.tensor.matmul(psa[:], lhsT=dm, rhs=g[:, j * H:j * H + 512],
                                 start=(j == 0), stop=(j == SPAN - 1))
                nc.tensor.matmul(psb[:], lhsT=dm,
                                 rhs=g[:, j * H + 512:(j + 1) * H],
                                 start=(j == 0), stop=(j == SPAN - 1))
            ct = gpool.tile([128, H], F16, tag=f"ct{h}", bufs=1)
            nc.vector.tensor_copy(ct[:, 0:512], psa[:])
            nc.vector.tensor_copy(ct[:, 512:768], psb[:])

            # center transposes (PE transpose mode, identity permutation)
            for cp in range(HC // 2):
                tpc = pst.tile([128, 256], F16, tag="tpc")
                for k in range(2):
                    c = 2 * cp + k
                    nc.tensor.transpose(tpc[:, k * 128:(k + 1) * 128],
                                        ct[:, c * 128:(c + 1) * 128], identity)
                nc.scalar.copy(
                    featT_pairs[:, 2 * cp:2 * cp + 2, h * 128:(h + 1) * 128],
                    tpc[:].rearrange("p (k x) -> p k x", k=2))

            # --- phase 4 (per half): center chunks close the accumulation ---
            for c in range(HC):
                for mt in range(2):
                    nc.tensor.matmul(
                        accs[mt][:, h * 128:(h + 1) * 128],
                        lhsT=wbT_t[:, c * INNER + mt * 128: c * INNER + (mt + 1) * 128],
                        rhs=featT[:, c * 256 + h * 128: c * 256 + h * 128 + 128],
                        start=False, stop=(c == HC - 1),
                    )
            for mt in range(2):
                nc.scalar.activation(
                    hiddenT[:, mt * 256 + h * 128: mt * 256 + h * 128 + 128],
                    accs[mt][:, h * 128:(h + 1) * 128],
                    mybir.ActivationFunctionType.Relu,
                    bias=bb_t[:, mt:mt + 1], scale=1.0)

        # --- phase 5: expert heads + per-sample selection ---
        out3 = pool.tile([128, 2 * NB_LABELS], F32)  # [p, h*3 + n]
        for h in range(2):
            b0 = h * 128
            mask36 = spool.tile([128, NE], F32, tag="mask36")
            nc.vector.tensor_scalar(mask36[:], io36f, catf[:, h:h + 1], None,
                                    op0=mybir.AluOpType.is_equal)
            ps36 = ps36p.tile([128, NE], F32, tag="ps36")
            nc.tensor.matmul(ps36[:], lhsT=hiddenT[:, b0:b0 + 128],
                             rhs=wexpA[:], start=True, stop=False)
            nc.tensor.matmul(ps36[:], lhsT=hiddenT[:, 256 + b0:256 + b0 + 128],
                             rhs=wexpB[:], start=False, stop=False)
            nc.tensor.matmul(ps36[:], lhsT=ones1[:, b0:b0 + 128],
                             rhs=wexpC[:], start=False, stop=True)

            prod = spool.tile([128, NE], F32, tag="prod")
            nc.vector.tensor_tensor(out=prod[:], in0=ps36[:], in1=mask36[:],
                                    op=mybir.AluOpType.mult)
            nc.vector.tensor_reduce(
                out=out3[:, h * NB_LABELS:(h + 1) * NB_LABELS],
                in_=prod[:].rearrange("p (e n) -> p n e", n=NB_LABELS),
                axis=mybir.AxisListType.X, op=mybir.AluOpType.add)
        nc.sync.dma_start(out.rearrange("(h p) n -> p h n", p=128),
                          out3[:].rearrange("p (h n) -> p h n", n=NB_LABELS))

    nc.compile()
    return nc


_NC = None


def _get_nc():
    global _NC
    if _NC is None:
        _NC = _build()
    return _NC


def _prep_inputs(embeddings, position_indexes, categories, W_base, b_base,
                 W_experts, b_experts):
    emb32 = np.asarray(embeddings)
    emb16 = emb32.astype(np.float16).reshape(NCORES, 2, HROWS, H)

    pos = np.asarray(position_indexes).astype(np.int64).reshape(NCORES, BC, 2)
    cat = np.asarray(categories).astype(np.int64).reshape(NCORES, BC)

    # gather start rows [128, 2]: col h = span-start row of sample h*128+p
    # within that half's emb tensor (8 contiguous rows always fit: spans lie
    # inside [0, S) and len <= 8).
    starts = pos[:, :, 0].reshape(NCORES, 2, 128)
    lens = (pos[:, :, 1] - pos[:, :, 0]).reshape(NCORES, 2, 128)
    base = np.arange(128, dtype=np.int64) * S
    rowA = base[None, None, :] + starts                       # [NC, 2, 128]
    gidx = rowA.transpose(0, 2, 1).reshape(NCORES, 128, 2).astype(np.int32)

    # dcst: diag(1/len) per half [0:256) + identity [256:384)
    rcp = 1.0 / lens.astype(np.float32)  # [NC, 2, 128]
    eye = np.eye(128, dtype=np.float32)
    diags = (eye[None, None] * rcp[:, :, :, None]).transpose(0, 2, 1, 3).reshape(
        NCORES, 128, 256)
    ident = np.broadcast_to(eye[None], (NCORES, 128, 128))
    dcst = np.concatenate([diags, ident], axis=2).astype(np.float16)

    # base linear: wbT_host[p, c*INNER+m] = W_base[m, c*128+p]
    wb = np.asarray(W_base, dtype=np.float32)  # [INNER, 3H]
    wbT = np.ascontiguousarray(
        wb.T.reshape(KC, 128, INNER).transpose(1, 0, 2).reshape(128, KC * INNER)
    ).astype(np.float16).reshape(F3H, INNER)

    bbias = np.ascontiguousarray(np.asarray(b_base, dtype=np.float32))

    we = np.asarray(W_experts, dtype=np.float32)  # [12, 3, INNER]
    be = np.asarray(b_experts, dtype=np.float32)  # [12, 3]
    wexpT = np.ascontiguousarray(np.concatenate(
        [we.transpose(2, 0, 1).reshape(INNER, NE), be.reshape(1, NE)],
        axis=0)).astype(np.float16)

    # static context rows, pre-transposed into featT layout:
    # ctxT[p, (which*6+cc)*256 + h*128 + sl] = emb[s(h,sl), CTX_IDX[which], cc*128+p]
    emb3 = emb16.reshape(NCORES, BC, S, H)
    blocks = []
    for which in range(NB_CTX):
        blk = emb3[:, :, CTX_IDX[which], :]                     # [NC, 256, 768]
        arr = blk.reshape(NCORES, 2, 128, HC, 128).transpose(0, 4, 3, 1, 2)
        blocks.append(arr.reshape(NCORES, 128, HC * 256))
    ctxT = np.ascontiguousarray(np.concatenate(blocks, axis=2))  # [NC, 128, 12*256]

    # io36 + categories-as-float + span masks (f32)
    cst32 = np.zeros((NCORES, 128, NE + 2 + 16), dtype=np.float32)
    cst32[:, :, :NE] = np.repeat(np.arange(NB_EXPERTS, dtype=np.float32),
                                 NB_LABELS)[None, None, :]
    cst32[:, :, NE:NE + 2] = cat.reshape(NCORES, 2, 128).transpose(0, 2, 1)
    j = np.arange(SPAN, dtype=np.int64)
    m8 = (j[None, None, None, :] < lens[..., None]).astype(np.float32)
    cst32[:, :, NE + 2:] = m8.transpose(0, 2, 1, 3).reshape(NCORES, 128, 16)

    onesd = np.ones((1, 256), dtype=np.float16)

    return [
        {"emb0": np.ascontiguousarray(emb16[i, 0]),
         "emb1": np.ascontiguousarray(emb16[i, 1]),
         "gidx": np.ascontiguousarray(gidx[i]),
         "wbT": wbT, "ctxT": ctxT[i], "dcst": np.ascontiguousarray(dcst[i]),
         "cst32": np.ascontiguousarray(cst32[i]), "bb": bbias,
         "wexpT": wexpT, "onesd": onesd}
        for i in range(NCORES)
    ]


def _run(in_maps, **kw):
    nc = _get_nc()
    return run_bass_kernel_spmd(nc, in_maps, core_ids=list(range(NCORES)), **kw)


def kernel(embeddings, position_indexes, categories, W_base, b_base, W_experts,
           b_experts):
    in_maps = _prep_inputs(embeddings, position_indexes, categories, W_base,
                           b_base, W_experts, b_experts)
    res = _run(in_maps)
    return np.concatenate([r["out"] for r in res.results], axis=0)


# revision 29
# speedup vs baseline: 1.3752x; 1.1399x over previous
"""Trainium2 Bass kernel for nn_BertClassifier_77309411685 (V7).

Data-parallel over 8 NeuronCores: each core handles 256 samples; the small
base linear and 12 expert heads are replicated.

Strategy:
  * fp16 end-to-end (host-cast); PSUM accumulation stays fp32.
  * span gather: ONE indirect DMA per half-batch of 128 samples — 8
    contiguous rows from `start` as a single 12KB descriptor per sample
    (spans always fit inside the sample's S rows, so no bounds handling).
    Rows past the span are real data, killed by masks later.  Single-index
    offset APs only: the HW DGE misreads multi-index offset APs.
  * masked mean on the PE: per half, 8 accumulating matmuls with stationary
    diag(mask_j/len) (built on DVE from a shipped diag(1/len) and 0/1 span
    masks) against the gathered rows -> center in PSUM, already averaged.
  * the 2 static context rows are host-sliced and shipped pre-transposed in
    featT layout, DMA'd straight into the featT tile.
  * base linear: ctx k-chunks run as soon as their weights land (DMA'd
    before the gathers queue); center k-chunks per half close the PSUM
    accumulation; bias+relu fused in the PSUM->SBUF activation.
  * expert heads: all 12 experts at once with bias folded via a ones row;
    per-sample selection by is_equal mask + strided reduce.
"""

import numpy as np
from contextlib import ExitStack

import concourse.bass as bass
import concourse.tile as tile
from concourse import bacc, mybir
from concourse.bass import IndirectOffsetOnAxis
from concourse.bass_utils import run_bass_kernel_spmd

F32 = mybir.dt.float32
F16 = mybir.dt.float16
I32 = mybir.dt.int32

B, S, H = 2048, 256, 768
INNER, NB_CTX, NB_EXPERTS, NB_LABELS = 256, 2, 12, 3
NCORES = 8
BC = B // NCORES             # 256 samples per core
F3H = (NB_CTX + 1) * H       # 2304
KC = F3H // 128              # 18 contraction chunks
HC = H // 128                # 6 chunks per feature block
NE = NB_EXPERTS * NB_LABELS  # 36
SPAN = 8
HROWS = 128 * S              # rows per half-batch tensor

# The reference picks 2 static context positions host-side with this exact rng.
CTX_IDX = [int(v) for v in np.random.default_rng(seed=0).choice(np.arange(S), size=NB_CTX)]




def _build():
    nc = bacc.Bacc(
        "TRN2",
        target_bir_lowering=False,
        debug=False,
        enable_asserts=False,
        num_devices=NCORES,
    )
    embs = [nc.dram_tensor(f"emb{h}", [HROWS, H], F16, kind="ExternalInput").ap()
            for h in range(2)]
    gidx = nc.dram_tensor("gidx", [128, 2], I32, kind="ExternalInput").ap()
    wbT = nc.dram_tensor("wbT", [F3H, INNER], F16, kind="ExternalInput").ap()
    ctxT = nc.dram_tensor("ctxT", [128, NB_CTX * HC * 256], F16, kind="ExternalInput").ap()
    # c16: diag(1/len) h0/h1 [0:256) + identity [256:384) + wexpA [384:420)
    #      + wexpB [420:456)
    c16 = nc.dram_tensor("c16", [128, 3 * 128 + 2 * NE], F16, kind="ExternalInput").ap()
    # c32: io36 [0:36) + categories-as-float [36:38) + span masks [38:54)
    #      + b_base (t p) layout [54:56)
    c32 = nc.dram_tensor("c32", [128, NE + 2 + 16 + 2], F32, kind="ExternalInput").ap()
    # c1: ones row [0:256) + wexp bias row [256:292)
    c1 = nc.dram_tensor("c1", [1, 256 + NE], F16, kind="ExternalInput").ap()
    out = nc.dram_tensor("out", [BC, NB_LABELS], F32, kind="ExternalOutput").ap()

    with tile.TileContext(nc) as tc, ExitStack() as ctx:
        pool = ctx.enter_context(tc.tile_pool(name="main", bufs=1))
        gpool = ctx.enter_context(tc.tile_pool(name="gp", bufs=2))
        spool = ctx.enter_context(tc.tile_pool(name="small", bufs=2))
        pst = ctx.enter_context(tc.tile_pool(name="pst", bufs=2, space="PSUM"))
        psh = ctx.enter_context(tc.tile_pool(name="psh", bufs=1, space="PSUM"))
        ps36p = ctx.enter_context(tc.tile_pool(name="ps36p", bufs=1, space="PSUM"))

        # --- phase 0: tiny front-of-queue loads the gathers depend on ---
        gidx_t = pool.tile([128, 2], I32)
        nc.sync.dma_start(gidx_t[:], gidx[:, :])

        # --- phase 1: span gathers — one op per half, 8 contiguous rows per
        # sample as a single 12KB descriptor (single-index offset AP only:
        # the HW DGE misreads multi-index offset APs) ---
        g_h = []
        for h in range(2):
            g = gpool.tile([128, SPAN * H], F16, tag=f"g{h}", bufs=1)
            nc.gpsimd.indirect_dma_start(
                out=g[:], out_offset=None, in_=embs[h],
                in_offset=IndirectOffsetOnAxis(ap=gidx_t[:, h:h + 1], axis=0),
            )
            g_h.append(g)

        # --- phase 2: big weight streams ride the ACT engine's separate
        # HWDGE queue so they start immediately; packed small consts follow
        # gidx on the sync queue ---
        featT = pool.tile([128, KC * 256], F16)
        wbT_t = pool.tile([128, KC * INNER], F16)
        wbT_c = wbT.rearrange("(p x) m -> p (x m)", p=128)
        # ctx-chunk weights first (phase 3b), then the static ctx rows
        # (pre-transposed in exact featT layout), center weights last
        nc.scalar.dma_start(wbT_t[:, HC * INNER:], wbT_c[:, HC * INNER:])
        nc.scalar.dma_start(featT[:, HC * 256:KC * 256], ctxT[:, :])
        nc.scalar.dma_start(wbT_t[:, :HC * INNER], wbT_c[:, :HC * INNER])

        c16_t = pool.tile([128, 3 * 128 + 2 * NE], F16)
        nc.sync.dma_start(c16_t[:], c16[:, :])
        dcst_t = c16_t  # diag/identity live in the packed blob
        wexpA = c16_t[:, 384:384 + NE]
        wexpB = c16_t[:, 384 + NE:384 + 2 * NE]
        c32_t = pool.tile([128, NE + 2 + 16 + 2], F32)
        nc.sync.dma_start(c32_t[:], c32[:, :])
        cst32_t = c32_t
        io36f = c32_t[:, 0:NE]
        catf = c32_t[:, NE:NE + 2]
        bb_t = c32_t[:, NE + 18:NE + 20]  # bb_t[p, t] = b_base[t*128 + p]
        c1_t = pool.tile([1, 256 + NE], F16)
        nc.sync.dma_start(c1_t[:], c1[:, :])
        ones1 = c1_t[:, 0:256]
        wexpC = c1_t[:, 256:256 + NE]

        # PE warm-up: the HAM clock gate releases after ~3.4us of sustained
        # activity; a burst of throwaway matmuls on the already-loaded const
        # tile warms the array before the real matmuls arrive.
        warm = pst.tile([128, 256], F32, tag="warm", bufs=1)
        for w in range(8):
            nc.tensor.matmul(warm[:], lhsT=c16_t[:, 0:128], rhs=c16_t[:, 0:256],
                             start=(w == 0), stop=(w == 7))

        # --- phase 3b/3c/4 interleaved per half ---
        # All base-linear matmuls are N=128, grouped per (m-tile, half): the
        # ctx chunks open each accumulation group (overlapping the gathers),
        # the center chunks close it.
        accs = [psh.tile([128, 256], F32, tag=f"acc{mt}", bufs=1, name=f"acc{mt}")
                for mt in range(2)]

        def ctx_mms(h):
            for c in range(HC, KC):
                for mt in range(2):
                    nc.tensor.matmul(
                        accs[mt][:, h * 128:(h + 1) * 128],
                        lhsT=wbT_t[:, c * INNER + mt * 128: c * INNER + (mt + 1) * 128],
                        rhs=featT[:, c * 256 + h * 128: c * 256 + h * 128 + 128],
                        start=(c == HC), stop=False,
                    )

        # diag(mask_j/len) stationaries for the PE mean: 8 per half, built
        # with cheap packed DVE tensor_scalar ops from diag(1/len) x m8[j].
        m8 = cst32_t[:, NE + 2:NE + 2 + 16]
        dmask = [pool.tile([128, SPAN * 128], F16, name=f"dmask{h}")
                 for h in range(2)]
        for h in range(2):
            diag = dcst_t[:, h * 128:(h + 1) * 128]
            for j in range(SPAN):
                nc.vector.tensor_scalar(
                    dmask[h][:, j * 128:(j + 1) * 128], diag,
                    m8[:, 8 * h + j:8 * h + j + 1], None,
                    op0=mybir.AluOpType.mult)

        featT_pairs = featT[:].rearrange("p (c x) -> p c x", x=256)
        hiddenT = pool.tile([128, 2 * 256], F16)
        identity = dcst_t[:, 256:384]
        for h in range(2):
            ctx_mms(h)
            g = g_h[h]
            # masked mean on the PE: center[q, x] = sum_j m_j[q]/len[q] *
            # g[q, j*H+x], accumulated in PSUM over the 8 slots.  Split at
            # 512 columns (PSUM bank limit).
            psa = pst.tile([128, 512], F32, tag="psa", bufs=1)
            psb = pst.tile([128, 256], F32, tag="psb", bufs=1)
            for j in range(SPAN):
                dm = dmask[h][:, j * 128:(j + 1) * 128]
                nc.tensor.matmul(psa[:], lhsT=dm, rhs=g[:, j * H:j * H + 512],
                                 start=(j == 0), stop=(j == SPAN - 1))
                nc.tensor.matmul(psb[:], lhsT=dm,
                                 rhs=g[:, j * H + 512:(j + 1) * H],
                                 start=(j == 0), stop=(j == SPAN - 1))
            ct = gpool.tile([128, H], F16, tag=f"ct{h}", bufs=1)
            nc.vector.tensor_copy(ct[:, 0:512], psa[:])
            nc.vector.tensor_copy(ct[:, 512:768], psb[:])

            # center transposes (PE transpose mode, identity permutation)
            for cp in range(HC // 2):
                tpc = pst.tile([128, 256], F16, tag="tpc")
                for k in range(2):
                    c = 2 * cp + k
                    nc.tensor.transpose(tpc[:, k * 128:(k + 1) * 128],
                                        ct[:, c * 128:(c + 1) * 128], identity)
                nc.scalar.copy(
                    featT_pairs[:, 2 * cp:2 * cp + 2, h * 128:(h + 1) * 128],
                    tpc[:].rearrange("p (k x) -> p k x", k=2))

            # --- phase 4 (per half): center chunks close the accumulation ---
            for c in range(HC):
                for mt in range(2):
                    nc.tensor.matmul(
                        accs[mt][:, h * 128:(h + 1) * 128],
                        lhsT=wbT_t[:, c * INNER + mt * 128: c * INNER + (mt + 1) * 128],
                        rhs=featT[:, c * 256 + h * 128: c * 256 + h * 128 + 128],
                        start=False, stop=(c == HC - 1),
                    )
            for mt in range(2):
                nc.scalar.activation(
                    hiddenT[:, mt * 256 + h * 128: mt * 256 + h * 128 + 128],
                    accs[mt][:, h * 128:(h + 1) * 128],
                    mybir.ActivationFunctionType.Relu,
                    bias=bb_t[:, mt:mt + 1], scale=1.0)

        # --- phase 5: expert heads + per-sample selection ---
        out3 = pool.tile([128, 2 * NB_LABELS], F32)  # [p, h*3 + n]
        for h in range(2):
            b0 = h * 128
            mask36 = spool.tile([128, NE], F32, tag="mask36")
            nc.vector.tensor_scalar(mask36[:], io36f, catf[:, h:h + 1], None,
                                    op0=mybir.AluOpType.is_equal)
            ps36 = ps36p.tile([128, NE], F32, tag="ps36")
            nc.tensor.matmul(ps36[:], lhsT=hiddenT[:, b0:b0 + 128],
                             rhs=wexpA, start=True, stop=False)
            nc.tensor.matmul(ps36[:], lhsT=hiddenT[:, 256 + b0:256 + b0 + 128],
                             rhs=wexpB, start=False, stop=False)
            nc.tensor.matmul(ps36[:], lhsT=ones1[0:1, b0:b0 + 128],
                             rhs=wexpC, start=False, stop=True)

            prod = spool.tile([128, NE], F32, tag="prod")
            nc.vector.tensor_tensor(out=prod[:], in0=ps36[:], in1=mask36[:],
                                    op=mybir.AluOpType.mult)
            nc.vector.tensor_reduce(
                out=out3[:, h * NB_LABELS:(h + 1) * NB_LABELS],
                in_=prod[:].rearrange("p (e n) -> p n e", n=NB_LABELS),
                axis=mybir.AxisListType.X, op=mybir.AluOpType.add)
        nc.sync.dma_start(out.rearrange("(h p) n -> p h n", p=128),
                          out3[:].rearrange("p (h n) -> p h n", n=NB_LABELS))

    nc.compile()
    return nc


_NC = None


def _get_nc():
    global _NC
    if _NC is None:
        _NC = _build()
    return _NC


def _prep_inputs(embeddings, position_indexes, categories, W_base, b_base,
                 W_experts, b_experts):
    emb32 = np.asarray(embeddings)
    emb16 = emb32.astype(np.float16).reshape(NCORES, 2, HROWS, H)

    pos = np.asarray(position_indexes).astype(np.int64).reshape(NCORES, BC, 2)
    cat = np.asarray(categories).astype(np.int64).reshape(NCORES, BC)

    # gather start rows [128, 2]: col h = span-start row of sample h*128+p
    # within that half's emb tensor (8 contiguous rows always fit: spans lie
    # inside [0, S) and len <= 8).
    starts = pos[:, :, 0].reshape(NCORES, 2, 128)
    lens = (pos[:, :, 1] - pos[:, :, 0]).reshape(NCORES, 2, 128)
    base = np.arange(128, dtype=np.int64) * S
    rowA = base[None, None, :] + starts                       # [NC, 2, 128]
    gidx = rowA.transpose(0, 2, 1).reshape(NCORES, 128, 2).astype(np.int32)

    rcp = 1.0 / lens.astype(np.float32)  # [NC, 2, 128]
    eye = np.eye(128, dtype=np.float32)
    diags = (eye[None, None] * rcp[:, :, :, None]).transpose(0, 2, 1, 3).reshape(
        NCORES, 128, 256)
    ident = np.broadcast_to(eye[None], (NCORES, 128, 128))

    # base linear: wbT_host[p, c*INNER+m] = W_base[m, c*128+p]
    wb = np.asarray(W_base, dtype=np.float32)  # [INNER, 3H]
    wbT = np.ascontiguousarray(
        wb.T.reshape(KC, 128, INNER).transpose(1, 0, 2).reshape(128, KC * INNER)
    ).astype(np.float16).reshape(F3H, INNER)

    bbias = np.ascontiguousarray(np.asarray(b_base, dtype=np.float32))

    we = np.asarray(W_experts, dtype=np.float32)  # [12, 3, INNER]
    be = np.asarray(b_experts, dtype=np.float32)  # [12, 3]
    wexp = we.transpose(2, 0, 1).reshape(INNER, NE)  # row m -> experts
    c16 = np.concatenate(
        [diags, ident,
         np.broadcast_to(wexp[None, 0:128], (NCORES, 128, NE)),
         np.broadcast_to(wexp[None, 128:256], (NCORES, 128, NE))],
        axis=2).astype(np.float16)
    c1 = np.concatenate(
        [np.ones((1, 256), dtype=np.float32), be.reshape(1, NE)],
        axis=1).astype(np.float16)

    # static context rows, pre-transposed into featT layout:
    # ctxT[p, (which*6+cc)*256 + h*128 + sl] = emb[s(h,sl), CTX_IDX[which], cc*128+p]
    emb3 = emb16.reshape(NCORES, BC, S, H)
    blocks = []
    for which in range(NB_CTX):
        blk = emb3[:, :, CTX_IDX[which], :]                     # [NC, 256, 768]
        arr = blk.reshape(NCORES, 2, 128, HC, 128).transpose(0, 4, 3, 1, 2)
        blocks.append(arr.reshape(NCORES, 128, HC * 256))
    ctxT = np.ascontiguousarray(np.concatenate(blocks, axis=2))  # [NC, 128, 12*256]

    # io36 + categories-as-float + span masks + b_base (f32)
    cst32 = np.zeros((NCORES, 128, NE + 2 + 16 + 2), dtype=np.float32)
    cst32[:, :, :NE] = np.repeat(np.arange(NB_EXPERTS, dtype=np.float32),
                                 NB_LABELS)[None, None, :]
    cst32[:, :, NE:NE + 2] = cat.reshape(NCORES, 2, 128).transpose(0, 2, 1)
    j = np.arange(SPAN, dtype=np.int64)
    m8 = (j[None, None, None, :] < lens[..., None]).astype(np.float32)
    cst32[:, :, NE + 2:NE + 18] = m8.transpose(0, 2, 1, 3).reshape(NCORES, 128, 16)
    cst32[:, :, NE + 18:] = bbias.reshape(2, 128).T[None]

    return [
        {"emb0": np.ascontiguousarray(emb16[i, 0]),
         "emb1": np.ascontiguousarray(emb16[i, 1]),
         "gidx": np.ascontiguousarray(gidx[i]),
         "wbT": wbT, "ctxT": ctxT[i],
         "c16": np.ascontiguousarray(c16[i]),
         "c32": np.ascontiguousarray(cst32[i]),
         "c1": np.ascontiguousarray(c1)}
        for i in range(NCORES)
    ]


def _run(in_maps, **kw):
    nc = _get_nc()
    return run_bass_kernel_spmd(nc, in_maps, core_ids=list(range(NCORES)), **kw)


def kernel(embeddings, position_indexes, categories, W_base, b_base, W_experts,
           b_experts):
    in_maps = _prep_inputs(embeddings, position_indexes, categories, W_base,
                           b_base, W_experts, b_experts)
    res = _run(in_maps)
    return np.concatenate([r["out"] for r in res.results], axis=0)


# revision 31
# speedup vs baseline: 1.4259x; 1.0368x over previous
"""Trainium2 Bass kernel for nn_BertClassifier_77309411685 (V7).

Data-parallel over 8 NeuronCores: each core handles 256 samples; the small
base linear and 12 expert heads are replicated.

Strategy:
  * fp16 end-to-end (host-cast); PSUM accumulation stays fp32.
  * span gather: ONE indirect DMA per half-batch of 128 samples — 8
    contiguous rows from `start` as a single 12KB descriptor per sample
    (spans always fit inside the sample's S rows, so no bounds handling).
    Rows past the span are real data, killed by masks later.  Single-index
    offset APs only: the HW DGE misreads multi-index offset APs.
  * masked mean on the PE: per half, 8 accumulating matmuls with stationary
    diag(mask_j/len) (built on DVE from a shipped diag(1/len) and 0/1 span
    masks) against the gathered rows -> center in PSUM, already averaged.
  * the 2 static context rows are host-sliced and shipped pre-transposed in
    featT layout, DMA'd straight into the featT tile.
  * base linear: ctx k-chunks run as soon as their weights land (DMA'd
    before the gathers queue); center k-chunks per half close the PSUM
    accumulation; bias+relu fused in the PSUM->SBUF activation.
  * expert heads: all 12 experts at once with bias folded via a ones row;
    per-sample selection by is_equal mask + strided reduce.
"""

import numpy as np
from contextlib import ExitStack

import concourse.bass as bass
import concourse.tile as tile
from concourse import bacc, mybir
from concourse.bass import IndirectOffsetOnAxis
from concourse.bass_utils import run_bass_kernel_spmd

F32 = mybir.dt.float32
F16 = mybir.dt.float16
I32 = mybir.dt.int32

B, S, H = 2048, 256, 768
INNER, NB_CTX, NB_EXPERTS, NB_LABELS = 256, 2, 12, 3
NCORES = 8
BC = B // NCORES             # 256 samples per core
F3H = (NB_CTX + 1) * H       # 2304
KC = F3H // 128              # 18 contraction chunks
HC = H // 128                # 6 chunks per feature block
NE = NB_EXPERTS * NB_LABELS  # 36
SPAN = 8
HROWS = 128 * S              # rows per half-batch tensor

# The reference picks 2 static context positions host-side with this exact rng.
CTX_IDX = [int(v) for v in np.random.default_rng(seed=0).choice(np.arange(S), size=NB_CTX)]




def _build():
    nc = bacc.Bacc(
        "TRN2",
        target_bir_lowering=False,
        debug=False,
        enable_asserts=False,
        num_devices=NCORES,
    )
    embs = [nc.dram_tensor(f"emb{h}", [HROWS, H], F16, kind="ExternalInput").ap()
            for h in range(2)]
    gidx = nc.dram_tensor("gidx", [128, 2], I32, kind="ExternalInput").ap()
    wbT = nc.dram_tensor("wbT", [F3H, INNER], F16, kind="ExternalInput").ap()
    ctxT = nc.dram_tensor("ctxT", [128, NB_CTX * HC * 256], F16, kind="ExternalInput").ap()
    # c16: diag(1/len) h0/h1 [0:256) + identity [256:384) + wexpA [384:420)
    #      + wexpB [420:456)
    c16 = nc.dram_tensor("c16", [128, 3 * 128 + 2 * NE], F16, kind="ExternalInput").ap()
    # c32: io36 [0:36) + categories-as-float [36:38) + span masks [38:54)
    #      + b_base (t p) layout [54:56)
    c32 = nc.dram_tensor("c32", [128, NE + 2 + 16 + 2], F32, kind="ExternalInput").ap()
    # c1: ones row [0:256) + wexp bias row [256:292)
    c1 = nc.dram_tensor("c1", [1, 256 + NE], F16, kind="ExternalInput").ap()
    out = nc.dram_tensor("out", [BC, NB_LABELS], F32, kind="ExternalOutput").ap()

    with tile.TileContext(nc) as tc, ExitStack() as ctx:
        pool = ctx.enter_context(tc.tile_pool(name="main", bufs=1))
        pst = ctx.enter_context(tc.tile_pool(name="pst", bufs=1, space="PSUM"))
        gpool = pool
        spool = pool
        psh = pst
        ps36p = pst

        # --- phase 0: tiny front-of-queue loads the gathers depend on ---
        gidx_t = pool.tile([128, 2], I32)
        nc.sync.dma_start(gidx_t[:], gidx[:, :])

        # --- phase 1: span gathers — one op per half, 8 contiguous rows per
        # sample as a single 12KB descriptor (single-index offset AP only:
        # the HW DGE misreads multi-index offset APs) ---
        g_h = []
        for h in range(2):
            g = gpool.tile([128, SPAN * H], F16, tag=f"g{h}", bufs=1)
            nc.gpsimd.indirect_dma_start(
                out=g[:], out_offset=None, in_=embs[h],
                in_offset=IndirectOffsetOnAxis(ap=gidx_t[:, h:h + 1], axis=0),
            )
            g_h.append(g)

        # --- phase 2: big weight streams ride the ACT engine's separate
        # HWDGE queue so they start immediately; packed small consts follow
        # gidx on the sync queue ---
        featT = pool.tile([128, KC * 256], F16)
        wbT_t = pool.tile([128, KC * INNER], F16)
        wbT_c = wbT.rearrange("(p x) m -> p (x m)", p=128)
        # ctx-chunk weights first (phase 3b), then the static ctx rows
        # (pre-transposed in exact featT layout), center weights last
        nc.scalar.dma_start(wbT_t[:, HC * INNER:], wbT_c[:, HC * INNER:])
        nc.scalar.dma_start(featT[:, HC * 256:KC * 256], ctxT[:, :])
        nc.scalar.dma_start(wbT_t[:, :HC * INNER], wbT_c[:, :HC * INNER])

        c16_t = pool.tile([128, 3 * 128 + 2 * NE], F16)
        nc.sync.dma_start(c16_t[:], c16[:, :])
        dcst_t = c16_t  # diag/identity live in the packed blob
        wexpA = c16_t[:, 384:384 + NE]
        wexpB = c16_t[:, 384 + NE:384 + 2 * NE]
        c32_t = pool.tile([128, NE + 2 + 16 + 2], F32)
        nc.sync.dma_start(c32_t[:], c32[:, :])
        cst32_t = c32_t
        io36f = c32_t[:, 0:NE]
        catf = c32_t[:, NE:NE + 2]
        bb_t = c32_t[:, NE + 18:NE + 20]  # bb_t[p, t] = b_base[t*128 + p]
        c1_t = pool.tile([1, 256 + NE], F16)
        nc.sync.dma_start(c1_t[:], c1[:, :])
        ones1 = c1_t[:, 0:256]
        wexpC = c1_t[:, 256:256 + NE]

        # PE warm-up: the HAM clock gate releases after ~3.4us of sustained
        # activity; a burst of throwaway matmuls on the already-loaded const
        # tile warms the array before the real matmuls arrive.
        warm = pst.tile([128, 256], F32, tag="psb0", bufs=1)
        for w in range(8):
            nc.tensor.matmul(warm[:], lhsT=c16_t[:, 0:128], rhs=c16_t[:, 0:256],
                             start=(w == 0), stop=(w == 7))

        # --- phase 3b/3c/4 interleaved per half ---
        # All base-linear matmuls are N=128, grouped per (m-tile, half): the
        # ctx chunks open each accumulation group (overlapping the gathers),
        # the center chunks close it.
        accs = [psh.tile([128, 256], F32, tag=f"acc{mt}", bufs=1, name=f"acc{mt}")
                for mt in range(2)]

        def ctx_mms(h):
            for c in range(HC, KC):
                for mt in range(2):
                    nc.tensor.matmul(
                        accs[mt][:, h * 128:(h + 1) * 128],
                        lhsT=wbT_t[:, c * INNER + mt * 128: c * INNER + (mt + 1) * 128],
                        rhs=featT[:, c * 256 + h * 128: c * 256 + h * 128 + 128],
                        start=(c == HC), stop=False,
                    )

        # diag(mask_j/len) stationaries for the PE mean: 8 per half, built
        # with cheap packed DVE tensor_scalar ops from diag(1/len) x m8[j].
        m8 = cst32_t[:, NE + 2:NE + 2 + 16]
        dmask = [pool.tile([128, SPAN * 128], F16, name=f"dmask{h}")
                 for h in range(2)]
        for h in range(2):
            diag = dcst_t[:, h * 128:(h + 1) * 128]
            for j in range(SPAN):
                nc.vector.tensor_scalar(
                    dmask[h][:, j * 128:(j + 1) * 128], diag,
                    m8[:, 8 * h + j:8 * h + j + 1], None,
                    op0=mybir.AluOpType.mult)

        featT_pairs = featT[:].rearrange("p (c x) -> p c x", x=256)
        hiddenT = pool.tile([128, 2 * 256], F16)
        identity = dcst_t[:, 256:384]
        for h in range(2):
            ctx_mms(h)
            g = g_h[h]
            # masked mean on the PE: center[q, x] = sum_j m_j[q]/len[q] *
            # g[q, j*H+x], accumulated in PSUM over the 8 slots.  Split at
            # 512 columns (PSUM bank limit).
            psa = pst.tile([128, 512], F32, tag=f"psa{h}", bufs=1)
            psb = pst.tile([128, 256], F32, tag=f"psb{h}", bufs=1, name=f"psb{h}")
            for j in range(SPAN):
                dm = dmask[h][:, j * 128:(j + 1) * 128]
                nc.tensor.matmul(psa[:], lhsT=dm, rhs=g[:, j * H:j * H + 512],
                                 start=(j == 0), stop=(j == SPAN - 1))
                nc.tensor.matmul(psb[:], lhsT=dm,
                                 rhs=g[:, j * H + 512:(j + 1) * H],
                                 start=(j == 0), stop=(j == SPAN - 1))
            ct = gpool.tile([128, H], F16, tag=f"ct{h}", bufs=1)
            nc.vector.tensor_copy(ct[:, 0:512], psa[:])
            nc.vector.tensor_copy(ct[:, 512:768], psb[:])

            # center transposes (PE transpose mode, identity permutation)
            for cp in range(HC // 2):
                tpc = pst.tile([128, 256], F16, tag=f"tpc{cp % 2}", bufs=1)
                for k in range(2):
                    c = 2 * cp + k
                    nc.tensor.transpose(tpc[:, k * 128:(k + 1) * 128],
                                        ct[:, c * 128:(c + 1) * 128], identity)
                nc.scalar.copy(
                    featT_pairs[:, 2 * cp:2 * cp + 2, h * 128:(h + 1) * 128],
                    tpc[:].rearrange("p (k x) -> p k x", k=2))

            # --- phase 4 (per half): center chunks close the accumulation ---
            for c in range(HC):
                for mt in range(2):
                    nc.tensor.matmul(
                        accs[mt][:, h * 128:(h + 1) * 128],
                        lhsT=wbT_t[:, c * INNER + mt * 128: c * INNER + (mt + 1) * 128],
                        rhs=featT[:, c * 256 + h * 128: c * 256 + h * 128 + 128],
                        start=False, stop=(c == HC - 1),
                    )
            for mt in range(2):
                nc.scalar.activation(
                    hiddenT[:, mt * 256 + h * 128: mt * 256 + h * 128 + 128],
                    accs[mt][:, h * 128:(h + 1) * 128],
                    mybir.ActivationFunctionType.Relu,
                    bias=bb_t[:, mt:mt + 1], scale=1.0)

        # --- phase 5: expert heads + per-sample selection ---
        out3 = pool.tile([128, 2 * NB_LABELS], F32)  # [p, h*3 + n]
        outv = out.rearrange("(h p) n -> p h n", p=128)
        for h in range(2):
            b0 = h * 128
            mask36 = spool.tile([128, NE], F32, tag=f"mask36{h}", bufs=1)
            nc.vector.tensor_scalar(mask36[:], io36f, catf[:, h:h + 1], None,
                                    op0=mybir.AluOpType.is_equal)
            ps36 = accs[h][:, 0:NE]
            nc.tensor.matmul(ps36, lhsT=hiddenT[:, b0:b0 + 128],
                             rhs=wexpA, start=True, stop=False)
            nc.tensor.matmul(ps36, lhsT=hiddenT[:, 256 + b0:256 + b0 + 128],
                             rhs=wexpB, start=False, stop=False)
            nc.tensor.matmul(ps36, lhsT=ones1[0:1, b0:b0 + 128],
                             rhs=wexpC, start=False, stop=True)

            prod = spool.tile([128, NE], F32, tag=f"prod{h}", bufs=1)
            nc.vector.tensor_tensor(out=prod[:], in0=ps36, in1=mask36[:],
                                    op=mybir.AluOpType.mult)
            nc.vector.tensor_reduce(
                out=out3[:, h * NB_LABELS:(h + 1) * NB_LABELS],
                in_=prod[:].rearrange("p (e n) -> p n e", n=NB_LABELS),
                axis=mybir.AxisListType.X, op=mybir.AluOpType.add)
            nc.sync.dma_start(
                outv[:, h:h + 1, :],
                out3[:].rearrange("p (g n) -> p g n", n=NB_LABELS)[:, h:h + 1, :])

    nc.compile()
    return nc


_NC = None


def _get_nc():
    global _NC
    if _NC is None:
        _NC = _build()
    return _NC


def _prep_inputs(embeddings, position_indexes, categories, W_base, b_base,
                 W_experts, b_experts):
    emb32 = np.asarray(embeddings)
    emb16 = emb32.astype(np.float16).reshape(NCORES, 2, HROWS, H)

    pos = np.asarray(position_indexes).astype(np.int64).reshape(NCORES, BC, 2)
    cat = np.asarray(categories).astype(np.int64).reshape(NCORES, BC)

    # gather start rows [128, 2]: col h = span-start row of sample h*128+p
    # within that half's emb tensor (8 contiguous rows always fit: spans lie
    # inside [0, S) and len <= 8).
    starts = pos[:, :, 0].reshape(NCORES, 2, 128)
    lens = (pos[:, :, 1] - pos[:, :, 0]).reshape(NCORES, 2, 128)
    base = np.arange(128, dtype=np.int64) * S
    rowA = base[None, None, :] + starts                       # [NC, 2, 128]
    gidx = rowA.transpose(0, 2, 1).reshape(NCORES, 128, 2).astype(np.int32)

    rcp = 1.0 / lens.astype(np.float32)  # [NC, 2, 128]
    eye = np.eye(128, dtype=np.float32)
    diags = (eye[None, None] * rcp[:, :, :, None]).transpose(0, 2, 1, 3).reshape(
        NCORES, 128, 256)
    ident = np.broadcast_to(eye[None], (NCORES, 128, 128))

    # base linear: wbT_host[p, c*INNER+m] = W_base[m, c*128+p]
    wb = np.asarray(W_base, dtype=np.float32)  # [INNER, 3H]
    wbT = np.ascontiguousarray(
        wb.T.reshape(KC, 128, INNER).transpose(1, 0, 2).reshape(128, KC * INNER)
    ).astype(np.float16).reshape(F3H, INNER)

    bbias = np.ascontiguousarray(np.asarray(b_base, dtype=np.float32))

    we = np.asarray(W_experts, dtype=np.float32)  # [12, 3, INNER]
    be = np.asarray(b_experts, dtype=np.float32)  # [12, 3]
    wexp = we.transpose(2, 0, 1).reshape(INNER, NE)  # row m -> experts
    c16 = np.concatenate(
        [diags, ident,
         np.broadcast_to(wexp[None, 0:128], (NCORES, 128, NE)),
         np.broadcast_to(wexp[None, 128:256], (NCORES, 128, NE))],
        axis=2).astype(np.float16)
    c1 = np.concatenate(
        [np.ones((1, 256), dtype=np.float32), be.reshape(1, NE)],
        axis=1).astype(np.float16)

    # static context rows, pre-transposed into featT layout:
    # ctxT[p, (which*6+cc)*256 + h*128 + sl] = emb[s(h,sl), CTX_IDX[which], cc*128+p]
    emb3 = emb16.reshape(NCORES, BC, S, H)
    blocks = []
    for which in range(NB_CTX):
        blk = emb3[:, :, CTX_IDX[which], :]                     # [NC, 256, 768]
        arr = blk.reshape(NCORES, 2, 128, HC, 128).transpose(0, 4, 3, 1, 2)
        blocks.append(arr.reshape(NCORES, 128, HC * 256))
    ctxT = np.ascontiguousarray(np.concatenate(blocks, axis=2))  # [NC, 128, 12*256]

    # io36 + categories-as-float + span masks + b_base (f32)
    cst32 = np.zeros((NCORES, 128, NE + 2 + 16 + 2), dtype=np.float32)
    cst32[:, :, :NE] = np.repeat(np.arange(NB_EXPERTS, dtype=np.float32),
                                 NB_LABELS)[None, None, :]
    cst32[:, :, NE:NE + 2] = cat.reshape(NCORES, 2, 128).transpose(0, 2, 1)
    j = np.arange(SPAN, dtype=np.int64)
    m8 = (j[None, None, None, :] < lens[..., None]).astype(np.float32)
    cst32[:, :, NE + 2:NE + 18] = m8.transpose(0, 2, 1, 3).reshape(NCORES, 128, 16)
    cst32[:, :, NE + 18:] = bbias.reshape(2, 128).T[None]

    return [
        {"emb0": np.ascontiguousarray(emb16[i, 0]),
         "emb1": np.ascontiguousarray(emb16[i, 1]),
         "gidx": np.ascontiguousarray(gidx[i]),
         "wbT": wbT, "ctxT": ctxT[i],
         "c16": np.ascontiguousarray(c16[i]),
         "c32": np.ascontiguousarray(cst32[i]),
         "c1": np.ascontiguousarray(c1)}
        for i in range(NCORES)
    ]


def _run(in_maps, **kw):
    nc = _get_nc()
    return run_bass_kernel_spmd(nc, in_maps, core_ids=list(range(NCORES)), **kw)


def kernel(embeddings, position_indexes, categories, W_base, b_base, W_experts,
           b_experts):
    in_maps = _prep_inputs(embeddings, position_indexes, categories, W_base,
                           b_base, W_experts, b_experts)
    res = _run(in_maps)
    return np.concatenate([r["out"] for r in res.results], axis=0)


# revision 35
# speedup vs baseline: 1.4579x; 1.0225x over previous
"""Trainium2 Bass kernel for nn_BertClassifier_77309411685 (V7).

Data-parallel over 8 NeuronCores: each core handles 256 samples; the small
base linear and 12 expert heads are replicated.

Strategy:
  * fp16 end-to-end (host-cast); PSUM accumulation stays fp32.
  * span gather: ONE indirect DMA per half-batch of 128 samples — 8
    contiguous rows from `start` as a single 12KB descriptor per sample
    (spans always fit inside the sample's S rows, so no bounds handling).
    Rows past the span are real data, killed by masks later.  Single-index
    offset APs only: the HW DGE misreads multi-index offset APs.
  * masked mean on the PE: per half, 8 accumulating matmuls with stationary
    diag(mask_j/len) (built on DVE from a shipped diag(1/len) and 0/1 span
    masks) against the gathered rows -> center in PSUM, already averaged.
  * the 2 static context rows are host-sliced and shipped pre-transposed in
    featT layout, DMA'd straight into the featT tile.
  * base linear: ctx k-chunks run as soon as their weights land (DMA'd
    before the gathers queue); center k-chunks per half close the PSUM
    accumulation; bias+relu fused in the PSUM->SBUF activation.
  * expert heads: all 12 experts at once with bias folded via a ones row;
    per-sample selection by is_equal mask + strided reduce.
"""

import numpy as np
from contextlib import ExitStack

import concourse.bass as bass
import concourse.tile as tile
from concourse import bacc, mybir
from concourse.bass import IndirectOffsetOnAxis
from concourse.bass_utils import run_bass_kernel_spmd

F32 = mybir.dt.float32
F16 = mybir.dt.float16
I32 = mybir.dt.int32

B, S, H = 2048, 256, 768
INNER, NB_CTX, NB_EXPERTS, NB_LABELS = 256, 2, 12, 3
NCORES = 8
BC = B // NCORES             # 256 samples per core
F3H = (NB_CTX + 1) * H       # 2304
KC = F3H // 128              # 18 contraction chunks
HC = H // 128                # 6 chunks per feature block
NE = NB_EXPERTS * NB_LABELS  # 36
SPAN = 8
HROWS = 128 * S              # rows per half-batch tensor

# The reference picks 2 static context positions host-side with this exact rng.
CTX_IDX = [int(v) for v in np.random.default_rng(seed=0).choice(np.arange(S), size=NB_CTX)]




def _build():
    nc = bacc.Bacc(
        "TRN2",
        target_bir_lowering=False,
        debug=False,
        enable_asserts=False,
        num_devices=NCORES,
    )
    embs = [nc.dram_tensor(f"emb{h}", [HROWS, H], F16, kind="ExternalInput").ap()
            for h in range(2)]
    gidx = nc.dram_tensor("gidx", [128, 2], I32, kind="ExternalInput").ap()
    wbT = nc.dram_tensor("wbT", [F3H, INNER], F16, kind="ExternalInput").ap()
    ctxT = nc.dram_tensor("ctxT", [128, NB_CTX * HC * 256], F16, kind="ExternalInput").ap()
    # c16: diag(1/len) h0/h1 [0:256) + identity [256:384) + wexpA [384:420)
    #      + wexpB [420:456)
    c16 = nc.dram_tensor("c16", [128, 3 * 128 + 2 * NE], F16, kind="ExternalInput").ap()
    # c32: io36 [0:36) + categories-as-float [36:38) + span masks [38:54)
    #      + b_base (t p) layout [54:56)
    c32 = nc.dram_tensor("c32", [128, NE + 2 + 16 + 2], F32, kind="ExternalInput").ap()
    # c1: ones row [0:256) + wexp bias row [256:292)
    c1 = nc.dram_tensor("c1", [1, 256 + NE], F16, kind="ExternalInput").ap()
    out = nc.dram_tensor("out", [BC, NB_LABELS], F32, kind="ExternalOutput").ap()

    with tile.TileContext(nc) as tc, ExitStack() as ctx:
        pool = ctx.enter_context(tc.tile_pool(name="main", bufs=1))
        pst = ctx.enter_context(tc.tile_pool(name="pst", bufs=1, space="PSUM"))
        gpool = pool
        spool = pool
        psh = pst
        ps36p = pst

        # --- phase 0: tiny front-of-queue loads the gathers depend on ---
        gidx_t = pool.tile([128, 2], I32)
        nc.sync.dma_start(gidx_t[:], gidx[:, :])

        # --- phase 1: span gathers — one op per half, 8 contiguous rows per
        # sample as a single 12KB descriptor (single-index offset AP only:
        # the HW DGE misreads multi-index offset APs) ---
        g_h = []
        for h in range(2):
            g = gpool.tile([128, SPAN * H], F16, tag=f"g{h}", bufs=1)
            nc.gpsimd.indirect_dma_start(
                out=g[:], out_offset=None, in_=embs[h],
                in_offset=IndirectOffsetOnAxis(ap=gidx_t[:, h:h + 1], axis=0),
            )
            g_h.append(g)

        # --- phase 2: big weight streams ride the ACT engine's separate
        # HWDGE queue so they start immediately; packed small consts follow
        # gidx on the sync queue ---
        featT = pool.tile([128, KC * 256], F16)
        wbT_t = pool.tile([128, KC * INNER], F16)
        wbT_c = wbT.rearrange("(p x) m -> p (x m)", p=128)
        # ctx-chunk weights first (phase 3b), then the static ctx rows
        # (pre-transposed in exact featT layout), center weights last
        nc.sync.dma_start(wbT_t[:, HC * INNER:], wbT_c[:, HC * INNER:])
        nc.sync.dma_start(featT[:, HC * 256:KC * 256], ctxT[:, :])
        nc.sync.dma_start(wbT_t[:, :HC * INNER], wbT_c[:, :HC * INNER])

        c16_t = pool.tile([128, 3 * 128 + 2 * NE], F16)
        nc.scalar.dma_start(c16_t[:], c16[:, :])
        dcst_t = c16_t  # diag/identity live in the packed blob
        wexpA = c16_t[:, 384:384 + NE]
        wexpB = c16_t[:, 384 + NE:384 + 2 * NE]
        c32_t = pool.tile([128, NE + 2 + 16 + 2], F32)
        nc.scalar.dma_start(c32_t[:], c32[:, :])
        cst32_t = c32_t
        io36f = c32_t[:, 0:NE]
        catf = c32_t[:, NE:NE + 2]
        bb_t = c32_t[:, NE + 18:NE + 20]  # bb_t[p, t] = b_base[t*128 + p]
        c1_t = pool.tile([1, 256 + NE], F16)
        nc.scalar.dma_start(c1_t[:], c1[:, :])
        ones1 = c1_t[:, 0:256]
        wexpC = c1_t[:, 256:256 + NE]

        # PE warm-up: the HAM clock gate releases after ~3.4us of sustained
        # activity; a burst of throwaway matmuls on the already-loaded const
        # tile warms the array before the real matmuls arrive.
        warm = pst.tile([128, 256], F32, tag="psb", bufs=1)
        for w in range(8):
            nc.tensor.matmul(warm[:], lhsT=c16_t[:, 0:128], rhs=c16_t[:, 0:256],
                             start=(w == 0), stop=(w == 7))

        # --- phase 3b/3c/4 interleaved per half ---
        # All base-linear matmuls are N=128, grouped per (m-tile, half): the
        # ctx chunks open each accumulation group (overlapping the gathers),
        # the center chunks close it.
        # one PSUM bank per (half, m-tile): all four base-linear accumulation
        # groups are open concurrently and a bank admits only one open group
        accs_h = [[psh.tile([128, 128], F32, tag=f"acc{h}{mt}", bufs=1,
                            name=f"acc{h}{mt}") for mt in range(2)]
                  for h in range(2)]

        def ctx_mms(h):
            for c in range(HC, KC):
                for mt in range(2):
                    nc.tensor.matmul(
                        accs_h[h][mt][:],
                        lhsT=wbT_t[:, c * INNER + mt * 128: c * INNER + (mt + 1) * 128],
                        rhs=featT[:, c * 256 + h * 128: c * 256 + h * 128 + 128],
                        start=(c == HC), stop=False,
                    )

        # diag(mask_j/len) stationaries for the PE mean: 8 per half, built
        # with cheap packed DVE tensor_scalar ops from diag(1/len) x m8[j].
        m8 = cst32_t[:, NE + 2:NE + 2 + 16]
        dmask = [pool.tile([128, SPAN * 128], F16, name=f"dmask{h}")
                 for h in range(2)]
        for h in range(2):
            diag = dcst_t[:, h * 128:(h + 1) * 128]
            for j in range(SPAN):
                nc.vector.tensor_scalar(
                    dmask[h][:, j * 128:(j + 1) * 128], diag,
                    m8[:, 8 * h + j:8 * h + j + 1], None,
                    op0=mybir.AluOpType.mult)

        featT_pairs = featT[:].rearrange("p (c x) -> p c x", x=256)
        hiddenT = pool.tile([128, 2 * 256], F16)
        identity = dcst_t[:, 256:384]
        out3 = pool.tile([128, 2 * NB_LABELS], F32)  # [p, h*3 + n]
        outv = out.rearrange("(h p) n -> p h n", p=128)

        ctx_mms(0)
        ctx_mms(1)

        # masked mean on the PE, both halves back to back so h1's matmuls
        # never queue behind h0's downstream chain
        ps_h = []
        for h in range(2):
            g = g_h[h]
            psa = pst.tile([128, 512], F32, tag=f"psa{h}", bufs=1)
            psb = pst.tile([128, 256], F32, tag="psb", bufs=1, name=f"psb{h}")
            for j in range(SPAN):
                dm = dmask[h][:, j * 128:(j + 1) * 128]
                nc.tensor.matmul(psa[:], lhsT=dm, rhs=g[:, j * H:j * H + 512],
                                 start=(j == 0), stop=(j == SPAN - 1))
                nc.tensor.matmul(psb[:], lhsT=dm,
                                 rhs=g[:, j * H + 512:(j + 1) * H],
                                 start=(j == 0), stop=(j == SPAN - 1))
            ps_h.append((psa, psb))

        for h in range(2):
            psa, psb = ps_h[h]
            ct = gpool.tile([128, H], F16, tag=f"ct{h}", bufs=1)
            nc.vector.tensor_copy(ct[:, 512:768], psb[:])
            nc.vector.tensor_copy(ct[:, 0:512], psa[:])

            # center transposes (PE transpose mode, identity permutation)
            for cp in range(HC // 2):
                tpc = pst.tile([128, 256], F16, tag="tpc", bufs=1)
                for k in range(2):
                    c = 2 * cp + k
                    nc.tensor.transpose(tpc[:, k * 128:(k + 1) * 128],
                                        ct[:, c * 128:(c + 1) * 128], identity)
                nc.scalar.copy(
                    featT_pairs[:, 2 * cp:2 * cp + 2, h * 128:(h + 1) * 128],
                    tpc[:].rearrange("p (k x) -> p k x", k=2))

            # center chunks close the base-linear accumulation; bias+relu
            for c in range(HC):
                for mt in range(2):
                    nc.tensor.matmul(
                        accs_h[h][mt][:],
                        lhsT=wbT_t[:, c * INNER + mt * 128: c * INNER + (mt + 1) * 128],
                        rhs=featT[:, c * 256 + h * 128: c * 256 + h * 128 + 128],
                        start=False, stop=(c == HC - 1),
                    )
            for mt in range(2):
                nc.scalar.activation(
                    hiddenT[:, mt * 256 + h * 128: mt * 256 + h * 128 + 128],
                    accs_h[h][mt][:],
                    mybir.ActivationFunctionType.Relu,
                    bias=bb_t[:, mt:mt + 1], scale=1.0)

            # expert heads + per-sample selection, inline per half
            b0 = h * 128
            mask36 = spool.tile([128, NE], F32, tag=f"mask36{h}", bufs=1)
            nc.vector.tensor_scalar(mask36[:], io36f, catf[:, h:h + 1], None,
                                    op0=mybir.AluOpType.is_equal)
            ps36 = ps_h[h][0][:, 0:NE]
            nc.tensor.matmul(ps36, lhsT=hiddenT[:, b0:b0 + 128],
                             rhs=wexpA, start=True, stop=False)
            nc.tensor.matmul(ps36, lhsT=hiddenT[:, 256 + b0:256 + b0 + 128],
                             rhs=wexpB, start=False, stop=False)
            nc.tensor.matmul(ps36, lhsT=ones1[0:1, b0:b0 + 128],
                             rhs=wexpC, start=False, stop=True)

            prod = spool.tile([128, NE], F32, tag=f"prod{h}", bufs=1)
            nc.vector.tensor_tensor(out=prod[:], in0=ps36, in1=mask36[:],
                                    op=mybir.AluOpType.mult)
            nc.vector.tensor_reduce(
                out=out3[:, h * NB_LABELS:(h + 1) * NB_LABELS],
                in_=prod[:].rearrange("p (e n) -> p n e", n=NB_LABELS),
                axis=mybir.AxisListType.X, op=mybir.AluOpType.add)
            nc.sync.dma_start(
                outv[:, h:h + 1, :],
                out3[:].rearrange("p (g n) -> p g n", n=NB_LABELS)[:, h:h + 1, :])

    nc.compile()
    return nc


_NC = None


def _get_nc():
    global _NC
    if _NC is None:
        _NC = _build()
    return _NC


def _prep_inputs(embeddings, position_indexes, categories, W_base, b_base,
                 W_experts, b_experts):
    emb32 = np.asarray(embeddings)
    emb16 = emb32.astype(np.float16).reshape(NCORES, 2, HROWS, H)

    pos = np.asarray(position_indexes).astype(np.int64).reshape(NCORES, BC, 2)
    cat = np.asarray(categories).astype(np.int64).reshape(NCORES, BC)

    # gather start rows [128, 2]: col h = span-start row of sample h*128+p
    # within that half's emb tensor (8 contiguous rows always fit: spans lie
    # inside [0, S) and len <= 8).
    starts = pos[:, :, 0].reshape(NCORES, 2, 128)
    lens = (pos[:, :, 1] - pos[:, :, 0]).reshape(NCORES, 2, 128)
    base = np.arange(128, dtype=np.int64) * S
    rowA = base[None, None, :] + starts                       # [NC, 2, 128]
    gidx = rowA.transpose(0, 2, 1).reshape(NCORES, 128, 2).astype(np.int32)

    rcp = 1.0 / lens.astype(np.float32)  # [NC, 2, 128]
    eye = np.eye(128, dtype=np.float32)
    diags = (eye[None, None] * rcp[:, :, :, None]).transpose(0, 2, 1, 3).reshape(
        NCORES, 128, 256)
    ident = np.broadcast_to(eye[None], (NCORES, 128, 128))

    # base linear: wbT_host[p, c*INNER+m] = W_base[m, c*128+p]
    wb = np.asarray(W_base, dtype=np.float32)  # [INNER, 3H]
    wbT = np.ascontiguousarray(
        wb.T.reshape(KC, 128, INNER).transpose(1, 0, 2).reshape(128, KC * INNER)
    ).astype(np.float16).reshape(F3H, INNER)

    bbias = np.ascontiguousarray(np.asarray(b_base, dtype=np.float32))

    we = np.asarray(W_experts, dtype=np.float32)  # [12, 3, INNER]
    be = np.asarray(b_experts, dtype=np.float32)  # [12, 3]
    wexp = we.transpose(2, 0, 1).reshape(INNER, NE)  # row m -> experts
    c16 = np.concatenate(
        [diags, ident,
         np.broadcast_to(wexp[None, 0:128], (NCORES, 128, NE)),
         np.broadcast_to(wexp[None, 128:256], (NCORES, 128, NE))],
        axis=2).astype(np.float16)
    c1 = np.concatenate(
        [np.ones((1, 256), dtype=np.float32), be.reshape(1, NE)],
        axis=1).astype(np.float16)

    # static context rows, pre-transposed into featT layout:
    # ctxT[p, (which*6+cc)*256 + h*128 + sl] = emb[s(h,sl), CTX_IDX[which], cc*128+p]
    emb3 = emb16.reshape(NCORES, BC, S, H)
    blocks = []
    for which in range(NB_CTX):
        blk = emb3[:, :, CTX_IDX[which], :]                     # [NC, 256, 768]
        arr = blk.reshape(NCORES, 2, 128, HC, 128).transpose(0, 4, 3, 1, 2)
        blocks.append(arr.reshape(NCORES, 128, HC * 256))
    ctxT = np.ascontiguousarray(np.concatenate(blocks, axis=2))  # [NC, 128, 12*256]

    # io36 + categories-as-float + span masks + b_base (f32)
    cst32 = np.zeros((NCORES, 128, NE + 2 + 16 + 2), dtype=np.float32)
    cst32[:, :, :NE] = np.repeat(np.arange(NB_EXPERTS, dtype=np.float32),
                                 NB_LABELS)[None, None, :]
    cst32[:, :, NE:NE + 2] = cat.reshape(NCORES, 2, 128).transpose(0, 2, 1)
    j = np.arange(SPAN, dtype=np.int64)
    m8 = (j[None, None, None, :] < lens[..., None]).astype(np.float32)
    cst32[:, :, NE + 2:NE + 18] = m8.transpose(0, 2, 1, 3).reshape(NCORES, 128, 16)
    cst32[:, :, NE + 18:] = bbias.reshape(2, 128).T[None]

    return [
        {"emb0": np.ascontiguousarray(emb16[i, 0]),
         "emb1": np.ascontiguousarray(emb16[i, 1]),
         "gidx": np.ascontiguousarray(gidx[i]),
         "wbT": wbT, "ctxT": ctxT[i],
         "c16": np.ascontiguousarray(c16[i]),
         "c32": np.ascontiguousarray(cst32[i]),
         "c1": np.ascontiguousarray(c1)}
        for i in range(NCORES)
    ]


def _run(in_maps, **kw):
    nc = _get_nc()
    return run_bass_kernel_spmd(nc, in_maps, core_ids=list(range(NCORES)), **kw)


def kernel(embeddings, position_indexes, categories, W_base, b_base, W_experts,
           b_experts):
    in_maps = _prep_inputs(embeddings, position_indexes, categories, W_base,
                           b_base, W_experts, b_experts)
    res = _run(in_maps)
    return np.concatenate([r["out"] for r in res.results], axis=0)
